# revision 21
# baseline (speedup 1.0000x reference)
"""Trainium2 Bass kernel for nn_Attention_82540681494971.

Spatial self-attention block (LDM AttnBlock style, unscaled):
  qkv = conv1x1(x);  s = q^T k  [n x n] per (b,head);  attn = softmax(s, axis=-1)
  out[d,m] = sum_n v[d,n] attn[n,m];  y = conv1x1(out)

Shapes: B=4, C=64, H=W=64 -> n=4096 tokens, HEAD=4, d=16.

Sharding: 8 cores, core c handles batch b=c//2 and heads (0,1) if c%2==0
else (2,3). Each core computes a partial projection output over its two
heads' channels; host sums the two partials per batch and adds proj bias.

Key algebra: attn[n,m] = E[n,m]/rowsum[n] with E=exp(s). Since the AV
contraction runs over n (the softmax row index), fold 1/rowsum into v:
  out[d,m] = sum_n (v[d,n]*rinv[n]) E[n,m]
so the big E matrix never needs normalizing.

Engine plan (ACT is the hard floor: 33.5M exp/core @ 1 elem/cyc/lane):
 - qkv: host pre-splits x and W into bf16 hi/lo; 2 bf16 matmuls per
   512-token chunk produce q,k,v for BOTH heads at once (M=96).
 - scores: exact-fp32-grade via a 3-term bf16 split (drop lo*lo):
     s = [q_hi;q_lo;q_hi]^T [k_hi;k_hi;k_lo]   (K=48 stacked)
 - AV: 8 persistent col-tiled accumulation chains per head live in 2
   PSUM banks for the whole head (tile_position cols 0/32/64/96); the
   first matmul of each bank is widened to M=112 with a zero-padded
   vts so the whole bank gets defined values + has_written bits.
 - exp: 3 chunked ACTIVATEs (1536/1536/1024) per 128-row block with
   accum_out giving the rowsum; ping-pong over 2x3 PSUM banks.
 - proj: outsb (bf16) stacked for both heads, single K=64 bf16 matmul.
"""

import numpy as np
import ml_dtypes
from contextlib import ExitStack

import concourse.bass as bass
import concourse.mybir as mybir
import concourse.tile as tile
from concourse import bacc
from concourse.bass import ts, ds
from concourse.bass_utils import run_bass_kernel_spmd

F32 = mybir.dt.float32
BF16 = mybir.dt.bfloat16
AF = mybir.ActivationFunctionType
BF = ml_dtypes.bfloat16

B, C, HEAD, D = 4, 64, 4, 16
N = 4096          # tokens = H*W
NT = 128          # n-tile (partition) size
NTILES = N // NT  # 32
MC = 512          # matmul free-dim chunk
SCH = (1536, 1536, 1024)  # scores/exp PSUM chunking


def _body(tc, y, t1, t2, whiB, whlB, wv1, wv2, wps0, wps1):
    nc = tc.nc
    ctx = ExitStack()
    with ctx:
        pp = ctx.enter_context(tc.tile_pool(name="persist", bufs=1))
        cp = ctx.enter_context(tc.tile_pool(name="consts", bufs=1))

        # ---- persistent SBUF ----
        t1_t = pp.tile([C + 1, N], BF16)   # [x_hi; ones]
        t2_t = pp.tile([2 * C, N], BF16)   # [x_lo; x_hi]
        # score stacks mirrored at partition 64: consecutive score matmuls
        # alternate PE row groups so each LDWEIGHTS hides under the other
        # group's stream (without this every matmul pays ldw serially).
        qsp = pp.tile([64 + 3 * D, 2 * N], BF16)  # [q_hi; q_lo; q_hi] x2
        ksp = pp.tile([64 + 3 * D, 2 * N], BF16)  # [k_hi; k_hi; k_lo] x2
        hi_t = pp.tile([96, N], BF16)      # bf16 of qkv psum (q0 q1 k0 k1 v0 v1)
        lo_t = pp.tile([C, N], BF16)       # residual for q,k rows
        vT = pp.tile([NT, 32 * NTILES], BF16)  # per n-tile [128, 32] (v h0|h1)
        # AV results, one [112, 1024] buffer per head; av tile ti -> cols
        # ti*512, m-chunk = 4*ti + strip/32 on partition rows strip..strip+16
        out_sb = [pp.tile([112, 1024], BF16, tag=f"osb{h}", name=f"osb{h}")
                  for h in range(2)]
        y_sb = pp.tile([C, N], F32)
        wtile = pp.tile([NT, MC], BF16)    # PE warm-up fodder

        # ---- constants ----
        whi_t = cp.tile([C + 1, 96], BF16)
        whl_t = cp.tile([2 * C, 96], BF16)
        wv1_t = cp.tile([C + 1, 32], BF16)   # [wv_lo; bv]
        wv2_t = cp.tile([2 * C, 32], BF16)   # [wv_hi; wv_hi]
        wps_t = [cp.tile([112, C], BF16, name=f"wps{h}") for h in range(2)]
        nc.gpsimd.memset(wtile[:], 0.0)
        for c in range(8):
            nc.sync.dma_start(t1_t[:, ts(c, MC)], t1[:, ts(c, MC)])
            nc.sync.dma_start(t2_t[:, ts(c, MC)], t2[:, ts(c, MC)])
            if c == 0:
                nc.sync.dma_start(whi_t[:], whiB[:])
                nc.sync.dma_start(whl_t[:], whlB[:])
                nc.sync.dma_start(wv1_t[:], wv1[:])
                nc.sync.dma_start(wv2_t[:], wv2[:])
        nc.sync.dma_start(wps_t[0][:], wps0[:])
        nc.sync.dma_start(wps_t[1][:], wps1[:])

        # ---- phase 0: fused qkv + bf16 hi/lo split ----
        with tc.tile_pool(name="p0psum", bufs=2, space="PSUM") as p0:
            # warm the PE's HAM clock gate while DMAs land: ~5us of dummy
            # matmuls with no DMA dependency (wtile is memset on-chip)
            for w in range(12):
                wps = p0.tile([NT, MC], F32, tag="warm", bufs=2,
                              name=f"warm{w}")
                nc.tensor.matmul(wps[:], wtile[:, ds(0, NT)], wtile[:],
                                 start=True, stop=True)
            for g in range(4):  # 1024-wide groups: fewer, larger evac ops
                ps = p0.tile([96, 2 * MC], F32, tag="p0", name=f"qkv{g}")
                for ci in range(2):
                    c = 2 * g + ci
                    nc.tensor.matmul(ps[:, ts(ci, MC)], whi_t[:],
                                     t1_t[:, ts(c, MC)], start=True, stop=False)
                    nc.tensor.matmul(ps[:, ts(ci, MC)], whl_t[:],
                                     t2_t[:, ts(c, MC)], start=False, stop=True)
                # hi on ACT (idle in phase 0), lo on DVE
                nc.scalar.copy(hi_t[:, ts(g, 2 * MC)], ps[:])
                nc.vector.tensor_sub(lo_t[:, ts(g, 2 * MC)], ps[ds(0, C), :],
                                     hi_t[ds(0, C), ts(g, 2 * MC)])
            # v for n-tiles 0-7 on the PE (token-partition form); only these
            # are needed early, the rest transpose in phase 1
            for nt in range(8):
                psv = p0.tile([NT, 32], F32, tag="pv", bufs=2,
                              name=f"psv{nt}")
                nc.tensor.matmul(psv[:], t2_t[:, ds(nt * NT, NT)],
                                 wv2_t[:], start=True, stop=False)
                nc.tensor.matmul(psv[:], t1_t[:, ds(nt * NT, NT)],
                                 wv1_t[:], start=False, stop=True)
                nc.vector.tensor_copy(vT[:, ts(nt, 32)], psv[:])

            # assembly into the mirrored K=48 score stacks (SBUF->SBUF DMA;
            # engines can't write partition base 16). DMA *dispatch* is
            # ~0.6us serial per op on a queue, so alternate between the two
            # hardware dispatch queues (sync + scalar).
            dq = [nc.sync, nc.scalar]
            di = 0
            for h in range(2):
                dsl = ds(h * N, N)
                sl = ds(0, N)
                qrow, krow = h * D, 32 + h * D
                for b0 in (0, 64):
                    for dst, src in (
                        (ksp[ds(b0, D), dsl], hi_t[ds(krow, D), sl]),
                        (ksp[ds(b0 + D, D), dsl], hi_t[ds(krow, D), sl]),
                        (ksp[ds(b0 + 2 * D, D), dsl], lo_t[ds(krow, D), sl]),
                        (qsp[ds(b0, D), dsl], hi_t[ds(qrow, D), sl]),
                        (qsp[ds(b0 + D, D), dsl], lo_t[ds(qrow, D), sl]),
                        (qsp[ds(b0 + 2 * D, D), dsl], hi_t[ds(qrow, D), sl]),
                    ):
                        dq[di % 2].dma_start(dst, src)
                        di += 1

        # ---- phase 1: attention, software-pipelined ----
        # Per step (h, nt): emit this n-tile's score matmuls + exp, woven
        # with the AV matmuls of the previous step. AV chains accumulate in
        # PSUM across the whole head (col-tiled 4-way, 2 banks per head).
        with (
            tc.tile_pool(name="ep", bufs=3) as ep,
            tc.tile_pool(name="rp", bufs=4) as rp,
            tc.tile_pool(name="vp", bufs=3) as vp,
            tc.tile_pool(name="sapsum", bufs=2, space="PSUM") as sp,
            tc.tile_pool(name="avpsum", bufs=1, space="PSUM") as avp,
        ):
            av_state = {}

            def emit_evac(h, engines):
                av_t, _ = av_state[h]
                for ti in range(2):
                    dst = out_sb[h][:, ts(ti, MC)]
                    if engines == "both" and ti == 1:
                        nc.scalar.copy(dst, av_t[ti][:])
                    else:
                        nc.vector.tensor_copy(dst, av_t[ti][:])

            prev = None
            for s in range(64):
                h, nt = divmod(s, NTILES)
                if nt == 0:
                    av_t = [avp.tile([112, MC], F32, tag=f"av{i}",
                                     name=f"av{i}h{h}") for i in range(2)]
                    vpad = vp.tile([NT, 112], BF16, tag="vpad", bufs=2,
                                   name=f"vpad{h}")
                    nc.gpsimd.memset(vpad[:], 0.0)
                    av_state[h] = (av_t, vpad)

                e_t = ep.tile([NT, N], BF16, tag="e", name=f"e{h}_{nt}")
                rsp = rp.tile([NT, 4], F32, tag="rs", name="rsp")

                chains = []
                if prev is not None:
                    ph, pnt, pe_t, pvts = prev
                    pav_t, pvpad = av_state[ph]

                    def mk(mc, pnt=pnt, pe_t=pe_t, pvts=pvts,
                           pav_t=pav_t, pvpad=pvpad):
                        def go():
                            ti, strip = mc // 4, 32 * (mc % 4)
                            if pnt == 0 and mc % 4 == 0:
                                # widened first matmul: writes the vts
                                # product on partitions 0-15 and zeros on
                                # 16-111, claiming the whole bank.
                                nc.tensor.matmul(
                                    pav_t[ti][:, :], pvpad[:, ds(0, 112)],
                                    pe_t[:, ts(mc, MC)],
                                    start=True, stop=False,
                                    skip_group_check=True)
                            else:
                                nc.tensor.matmul(
                                    pav_t[ti][ds(strip, D), :], pvts,
                                    pe_t[:, ts(mc, MC)],
                                    start=False, stop=(pnt == NTILES - 1),
                                    tile_position=(0, strip),
                                    skip_group_check=True)
                        return go

                    chains = [mk(mc) for mc in range(8)]

                # weave: score chunks c0+c1 first (they gate the EXP chain),
                # then the prev step's AV block (gated by its vts), then c2.
                off = 0
                for ci, csz in enumerate(SCH):
                    s_ps = sp.tile([NT, SCH[0]], F32, tag="sa", name="s_ps")
                    for i in range(csz // MC):
                        b0 = 64 if (off // MC + i) % 2 else 0
                        nc.tensor.matmul(
                            s_ps[:, ts(i, MC)],
                            qsp[ds(b0, 3 * D), ds(h * N + nt * NT, NT)],
                            ksp[ds(b0, 3 * D), ds(h * N + off + i * MC, MC)],
                            start=True, stop=True)
                    nc.scalar.activation(
                        e_t[:, ds(off, csz)], s_ps[:, ds(0, csz)],
                        AF.Exp, accum_out=rsp[:, ds(ci, 1)])
                    off += csz
                    if ci == 1:
                        for _ in range(6):
                            if chains:
                                chains.pop(0)()
                while chains:
                    chains.pop(0)()
                if s < 24:
                    # v for n-tile s+8 via xbar transpose (serial on the
                    # sync engine, ~1.2us each -- one per step is free here)
                    nc.sync.dma_start(vT[:, ts(s + 8, 32)],
                                      hi_t[ds(64, 32), ts(s + 8, NT)],
                                      transpose=True)

                rs = rp.tile([NT, 1], F32, tag="r1", name="rs1")
                rinv = rp.tile([NT, 1], F32, tag="ri", name="rinv")
                nc.vector.reduce_sum(rs[:], rsp[:, ds(0, 3)],
                                     axis=mybir.AxisListType.X)
                nc.vector.reciprocal(rinv[:], rs[:])
                if nt == 0:
                    vts = av_state[h][1][:, ds(0, D)]
                else:
                    vts_t = vp.tile([NT, D], BF16, tag="vts",
                                    name=f"vts{h}_{nt}")
                    vts = vts_t[:]
                nc.vector.tensor_scalar_mul(
                    vts, vT[:, ds(nt * 32 + h * D, D)], rinv[:])
                prev = (h, nt, e_t, vts)

                if s == NTILES:
                    # AV(h0, 31) was just woven above; drain head-0 chains
                    emit_evac(0, "vector")

            # ---- tail: flush AV(h1, 31), drain, project ----
            ph, pnt, pe_t, pvts = prev
            pav_t, _ = av_state[ph]
            for mc in range(8):
                ti, strip = mc // 4, 32 * (mc % 4)
                nc.tensor.matmul(
                    pav_t[ti][ds(strip, D), :], pvts, pe_t[:, ts(mc, MC)],
                    start=False, stop=True, tile_position=(0, strip),
                    skip_group_check=True)
            emit_evac(1, "both")

            # y[:, mc] = sum_h wps_h^T out_h[:, mc]; wps replicas at every
            # 32-strip let lhsT/rhs share the strip's base partition.
            # yp rotates through 4 PSUM slots (sa x2 + the freed av banks)
            # so the MM -> copy -> DMA chain pipelines 4 deep.
            for mc in range(8):
                ti, strip = mc // 4, 32 * (mc % 4)
                tag = ("sa", "av0", "av1")[mc % 3]
                yp = sp.tile([C, MC], F32, tag="sa", name=f"yp{mc}") \
                    if tag == "sa" else \
                    avp.tile([C, MC], F32, tag=tag, name=f"yp{mc}")
                for hh in range(2):
                    nc.tensor.matmul(
                        yp[:], wps_t[hh][ds(strip, D), :],
                        out_sb[hh][ds(strip, D), ts(ti, MC)],
                        start=(hh == 0), stop=(hh == 1),
                        tile_position=(strip, 0))
                if mc % 2 == 0:
                    nc.vector.tensor_copy(y_sb[:, ts(mc, MC)], yp[:])
                else:
                    nc.scalar.copy(y_sb[:, ts(mc, MC)], yp[:])
                dq2 = nc.sync if mc % 2 == 0 else nc.scalar
                dq2.dma_start(y[:, ts(mc, MC)], y_sb[:, ts(mc, MC)])


_PROGRAM = None


def _get_program():
    global _PROGRAM
    if _PROGRAM is None:
        nc = bacc.Bacc("TRN2", target_bir_lowering=False, debug=False,
                       num_devices=8)
        t1 = nc.dram_tensor("t1", [C + 1, N], BF16, kind="ExternalInput").ap()
        t2 = nc.dram_tensor("t2", [2 * C, N], BF16, kind="ExternalInput").ap()
        whiB = nc.dram_tensor("whiB", [C + 1, 96], BF16, kind="ExternalInput").ap()
        whlB = nc.dram_tensor("whlB", [2 * C, 96], BF16, kind="ExternalInput").ap()
        wv1 = nc.dram_tensor("wv1", [C + 1, 32], BF16, kind="ExternalInput").ap()
        wv2 = nc.dram_tensor("wv2", [2 * C, 32], BF16, kind="ExternalInput").ap()
        wps0 = nc.dram_tensor("wps0", [112, C], BF16, kind="ExternalInput").ap()
        wps1 = nc.dram_tensor("wps1", [112, C], BF16, kind="ExternalInput").ap()
        y = nc.dram_tensor("y", [C, N], F32, kind="ExternalOutput").ap()
        with tile.TileContext(nc) as tc:
            _body(tc, y, t1, t2, whiB, whlB, wv1, wv2, wps0, wps1)
        nc.compile()
        _PROGRAM = nc
    return _PROGRAM


def _make_in_maps(x, qkv_w, qkv_b, proj_w, proj_b=None):
    x = np.asarray(x, dtype=np.float32)
    qkv_w = np.asarray(qkv_w, dtype=np.float32)
    qkv_b = np.asarray(qkv_b, dtype=np.float32)
    proj_w = np.asarray(proj_w, dtype=np.float32)

    # per-batch tensors (shared by the 2 cores of a batch)
    t1s, t2s = [], []
    for b in range(B):
        xf = x[b].reshape(C, N)
        xh = xf.astype(BF)
        xl = (xf - xh.astype(np.float32)).astype(BF)
        t1s.append(np.ascontiguousarray(
            np.concatenate([xh, np.ones((1, N), BF)], axis=0)))
        t2s.append(np.ascontiguousarray(np.concatenate([xl, xh], axis=0)))

    in_maps = []
    for core in range(8):
        b = core // 2
        h0 = 2 * (core % 2)
        heads = (h0, h0 + 1)

        # weight stack cols: [q_h0, q_h1, k_h0, k_h1, v_h0, v_h1]
        rows = []
        bias = []
        for blk in range(3):
            for h in heads:
                r = slice(blk * C + h * D, blk * C + (h + 1) * D)
                rows.append(qkv_w[r, :])
                bias.append(qkv_b[r])
        Wsel = np.concatenate(rows, axis=0)          # [96, 64]
        bsel = np.concatenate(bias, axis=0)          # [96]
        Whi = Wsel.astype(BF)
        Wlo = (Wsel - Whi.astype(np.float32)).astype(BF)
        whiB = np.zeros((C + 1, 96), BF)
        whiB[:C] = Whi.T
        whiB[C] = bsel.astype(BF)
        whlB = np.ascontiguousarray(
            np.concatenate([Whi.T.astype(BF), Wlo.T.astype(BF)], axis=0))
        # v weights for the early token-partition v matmuls (cols 64-95 of
        # the stack are the v heads)
        wv1B = np.zeros((C + 1, 32), BF)
        wv1B[:C] = Wlo.T[:, 64:96]
        wv1B[C] = bsel[64:96].astype(BF)
        wv2B = np.ascontiguousarray(
            np.concatenate([Whi.T[:, 64:96], Whi.T[:, 64:96]], axis=0))

        # proj weights, replicated at every 32-partition strip so the proj
        # matmul's lhsT base partition matches its rhs strip
        wpss = []
        for h in heads:
            w = np.zeros((112, C), BF)
            blk = proj_w[:, h * D:(h + 1) * D].T.astype(BF)
            for strip in range(4):
                w[strip * 32:strip * 32 + D, :] = blk
            wpss.append(w)

        in_maps.append({
            "t1": t1s[b],
            "t2": t2s[b],
            "whiB": whiB,
            "whlB": whlB,
            "wv1": wv1B,
            "wv2": wv2B,
            "wps0": wpss[0],
            "wps1": wpss[1],
        })
    return in_maps


def run_cores(inputs, **kw):
    """Compile+run on the 8 cores; returns BassKernelResults."""
    nc = _get_program()
    in_maps = _make_in_maps(**inputs)
    return run_bass_kernel_spmd(nc, in_maps, list(range(8)), **kw)


def kernel(x, qkv_w, qkv_b, proj_w, proj_b):
    res = run_cores(dict(x=x, qkv_w=qkv_w, qkv_b=qkv_b,
                         proj_w=proj_w, proj_b=proj_b))
    proj_b = np.asarray(proj_b, dtype=np.float32)
    parts = [np.asarray(r["y"], dtype=np.float32) for r in res.results]
    out = np.empty((B, C, N), np.float32)
    for b in range(B):
        out[b] = parts[2 * b] + parts[2 * b + 1] + proj_b[:, None]
    return out.reshape(B, C, 64, 64)


if __name__ == "__main__":
    _get_program()
    print("program built OK")


# revision 25
# speedup vs baseline: 1.0389x; 1.0389x over previous
"""Trainium2 Bass kernel for nn_Attention_82540681494971.

Spatial self-attention block (LDM AttnBlock style, unscaled):
  qkv = conv1x1(x);  s = q^T k  [n x n] per (b,head);  attn = softmax(s, axis=-1)
  out[d,m] = sum_n v[d,n] attn[n,m];  y = conv1x1(out)

Shapes: B=4, C=64, H=W=64 -> n=4096 tokens, HEAD=4, d=16.

Sharding: 8 cores, core c handles batch b=c//2 and heads (0,1) if c%2==0
else (2,3). Each core computes a partial projection output over its two
heads' channels; host sums the two partials per batch and adds proj bias.

Key algebra: attn[n,m] = E[n,m]/rowsum[n] with E=exp(s). Since the AV
contraction runs over n (the softmax row index), fold 1/rowsum into v:
  out[d,m] = sum_n (v[d,n]*rinv[n]) E[n,m]
so the big E matrix never needs normalizing.

Engine plan (ACT is the hard floor: 33.5M exp/core @ 1 elem/cyc/lane):
 - qkv: host pre-splits x and W into bf16 hi/lo; 2 bf16 matmuls per
   512-token chunk produce q,k,v for BOTH heads at once (M=96).
 - scores: exact-fp32-grade via a 3-term bf16 split (drop lo*lo):
     s = [q_hi;q_lo;q_hi]^T [k_hi;k_hi;k_lo]   (K=48 stacked)
 - AV: 8 persistent col-tiled accumulation chains per head live in 2
   PSUM banks for the whole head (tile_position cols 0/32/64/96); the
   first matmul of each bank is widened to M=112 with a zero-padded
   vts so the whole bank gets defined values + has_written bits.
 - exp: 3 chunked ACTIVATEs (1536/1536/1024) per 128-row block with
   accum_out giving the rowsum; ping-pong over 2x3 PSUM banks.
 - proj: outsb (bf16) stacked for both heads, single K=64 bf16 matmul.
"""

import numpy as np
import ml_dtypes
from contextlib import ExitStack

import concourse.bass as bass
import concourse.mybir as mybir
import concourse.tile as tile
from concourse import bacc
from concourse.bass import ts, ds
from concourse.bass_utils import run_bass_kernel_spmd

F32 = mybir.dt.float32
BF16 = mybir.dt.bfloat16
AF = mybir.ActivationFunctionType
BF = ml_dtypes.bfloat16

B, C, HEAD, D = 4, 64, 4, 16
N = 4096          # tokens = H*W
NT = 128          # n-tile (partition) size
NTILES = N // NT  # 32
MC = 512          # matmul free-dim chunk
SCH = (1024, 1536, 1536)  # scores/exp PSUM chunking (small chunk first so
                          # the first EXP needs only one assembled quarter)


def _body(tc, y, t1, t2, whiB, whlB, wv1, wv2, wps0, wps1):
    nc = tc.nc
    ctx = ExitStack()
    with ctx:
        pp = ctx.enter_context(tc.tile_pool(name="persist", bufs=1))
        cp = ctx.enter_context(tc.tile_pool(name="consts", bufs=1))

        # ---- persistent SBUF ----
        t1_t = pp.tile([C + 1, N], BF16)   # [x_hi; ones]
        t2_t = pp.tile([2 * C, N], BF16)   # [x_lo; x_hi]
        # score stacks mirrored at partition 64: consecutive score matmuls
        # alternate PE row groups so each LDWEIGHTS hides under the other
        # group's stream (without this every matmul pays ldw serially).
        qsp = pp.tile([64 + 3 * D, 2 * N], BF16)  # [q_hi; q_lo; q_hi] x2
        ksp = pp.tile([64 + 3 * D, 2 * N], BF16)  # [k_hi; k_hi; k_lo] x2
        hi_t = pp.tile([96, N], BF16)      # bf16 of qkv psum (q0 q1 k0 k1 v0 v1)
        lo_t = pp.tile([C, N], BF16)       # residual for q,k rows
        vT = pp.tile([NT, 32 * NTILES], BF16)  # per n-tile [128, 32] (v h0|h1)
        # AV results, one [112, 1024] buffer per head; av tile ti -> cols
        # ti*512, m-chunk = 4*ti + strip/32 on partition rows strip..strip+16
        out_sb = [pp.tile([112, 1024], BF16, tag=f"osb{h}", name=f"osb{h}")
                  for h in range(2)]
        y_sb = pp.tile([C, N], F32)
        wtile = pp.tile([NT, MC], BF16)    # PE warm-up fodder

        # ---- constants ----
        whi_t = cp.tile([C + 1, 96], BF16)
        whl_t = cp.tile([2 * C, 96], BF16)
        wv1_t = cp.tile([C + 1, 32], BF16)   # [wv_lo; bv]
        wv2_t = cp.tile([2 * C, 32], BF16)   # [wv_hi; wv_hi]
        wps_t = [cp.tile([112, C], BF16, name=f"wps{h}") for h in range(2)]
        nc.gpsimd.memset(wtile[:], 0.0)
        for c in range(8):
            nc.sync.dma_start(t1_t[:, ts(c, MC)], t1[:, ts(c, MC)])
            nc.sync.dma_start(t2_t[:, ts(c, MC)], t2[:, ts(c, MC)])
            if c == 0:
                nc.sync.dma_start(whi_t[:], whiB[:])
                nc.sync.dma_start(whl_t[:], whlB[:])
                nc.sync.dma_start(wv1_t[:], wv1[:])
                nc.sync.dma_start(wv2_t[:], wv2[:])
        nc.sync.dma_start(wps_t[0][:], wps0[:])
        nc.sync.dma_start(wps_t[1][:], wps1[:])

        # ---- phase 0: fused qkv + bf16 hi/lo split ----
        with tc.tile_pool(name="p0psum", bufs=2, space="PSUM") as p0:
            # warm the PE's HAM clock gate while DMAs land: ~5us of dummy
            # matmuls with no DMA dependency (wtile is memset on-chip)
            for w in range(12):
                wps = p0.tile([NT, MC], F32, tag="warm", bufs=2,
                              name=f"warm{w}")
                nc.tensor.matmul(wps[:], wtile[:, ds(0, NT)], wtile[:],
                                 start=True, stop=True)
            for g in range(4):  # 1024-wide groups: fewer, larger evac ops
                ps = p0.tile([96, 2 * MC], F32, tag="p0", name=f"qkv{g}")
                for ci in range(2):
                    c = 2 * g + ci
                    nc.tensor.matmul(ps[:, ts(ci, MC)], whi_t[:],
                                     t1_t[:, ts(c, MC)], start=True, stop=False)
                    nc.tensor.matmul(ps[:, ts(ci, MC)], whl_t[:],
                                     t2_t[:, ts(c, MC)], start=False, stop=True)
                # hi on ACT (idle in phase 0), lo on DVE
                nc.scalar.copy(hi_t[:, ts(g, 2 * MC)], ps[:])
                nc.vector.tensor_sub(lo_t[:, ts(g, 2 * MC)], ps[ds(0, C), :],
                                     hi_t[ds(0, C), ts(g, 2 * MC)])
            # v for n-tiles 0-7 on the PE (token-partition form); only these
            # are needed early, the rest transpose in phase 1
            for nt in range(8):
                psv = p0.tile([NT, 32], F32, tag="pv", bufs=2,
                              name=f"psv{nt}")
                nc.tensor.matmul(psv[:], t2_t[:, ds(nt * NT, NT)],
                                 wv2_t[:], start=True, stop=False)
                nc.tensor.matmul(psv[:], t1_t[:, ds(nt * NT, NT)],
                                 wv1_t[:], start=False, stop=True)
                nc.vector.tensor_copy(vT[:, ts(nt, 32)], psv[:])

            # assembly into the mirrored K=48 score stacks (SBUF->SBUF DMA;
            # engines can't write partition base 16). SBUF->SBUF DMA runs at
            # only ~110GB/s serially on the dispatching queue, so it can't be
            # made fast -- it must be HIDDEN: emit in dependency-priority
            # order (k h0 quarter 0 first, unblocking the first EXPs) and
            # let the rest stream in behind the early attention steps.
            def asm(t, h, qt):
                dsl = ds(h * N + qt * 1024, 1024)
                sl = ds(qt * 1024, 1024)
                if t == "k":
                    row = 32 + h * D
                    blocks = ((0, hi_t), (D, hi_t), (2 * D, lo_t))
                else:
                    row = h * D
                    blocks = ((0, hi_t), (D, lo_t), (2 * D, hi_t))
                dst = ksp if t == "k" else qsp
                for b0 in (0, 64):
                    for boff, src in blocks:
                        nc.sync.dma_start(dst[ds(b0 + boff, D), dsl],
                                          src[ds(row, D), sl])

            asm("k", 0, 0)
            asm("q", 0, 0)
            for qt in (1, 2, 3):
                asm("k", 0, qt)
            asm("q", 0, 1)
            for qt in range(4):
                asm("k", 1, qt)
            # v transposes for n-tiles 8-15 (rest are woven into phase 1)
            for nt in range(8, 16):
                nc.sync.dma_start(vT[:, ts(nt, 32)],
                                  hi_t[ds(64, 32), ts(nt, NT)],
                                  transpose=True)
            asm("q", 0, 2)
            asm("q", 0, 3)
            for qt in range(4):
                asm("q", 1, qt)

        # ---- phase 1: attention, software-pipelined ----
        # Per step (h, nt): emit this n-tile's score matmuls + exp, woven
        # with the AV matmuls of the previous step. AV chains accumulate in
        # PSUM across the whole head (col-tiled 4-way, 2 banks per head).
        with (
            tc.tile_pool(name="ep", bufs=3) as ep,
            tc.tile_pool(name="rp", bufs=4) as rp,
            tc.tile_pool(name="vp", bufs=3) as vp,
            tc.tile_pool(name="sapsum", bufs=2, space="PSUM") as sp,
            tc.tile_pool(name="avpsum", bufs=1, space="PSUM") as avp,
        ):
            av_state = {}

            def emit_evac(h, engines):
                av_t, _ = av_state[h]
                for ti in range(2):
                    dst = out_sb[h][:, ts(ti, MC)]
                    if engines == "both" and ti == 1:
                        nc.scalar.copy(dst, av_t[ti][:])
                    else:
                        nc.vector.tensor_copy(dst, av_t[ti][:])

            prev = None
            for s in range(64):
                h, nt = divmod(s, NTILES)
                if nt == 0:
                    av_t = [avp.tile([112, MC], F32, tag=f"av{i}",
                                     name=f"av{i}h{h}") for i in range(2)]
                    vpad = vp.tile([NT, 112], BF16, tag="vpad", bufs=2,
                                   name=f"vpad{h}")
                    nc.gpsimd.memset(vpad[:], 0.0)
                    av_state[h] = (av_t, vpad)

                e_t = ep.tile([NT, N], BF16, tag="e", name=f"e{h}_{nt}")
                rsp = rp.tile([NT, 4], F32, tag="rs", name="rsp")

                chains = []
                if prev is not None:
                    ph, pnt, pe_t, pvts = prev
                    pav_t, pvpad = av_state[ph]

                    def mk(mc, pnt=pnt, pe_t=pe_t, pvts=pvts,
                           pav_t=pav_t, pvpad=pvpad):
                        def go():
                            ti, strip = mc // 4, 32 * (mc % 4)
                            if pnt == 0 and mc % 4 == 0:
                                # widened first matmul: writes the vts
                                # product on partitions 0-15 and zeros on
                                # 16-111, claiming the whole bank.
                                nc.tensor.matmul(
                                    pav_t[ti][:, :], pvpad[:, ds(0, 112)],
                                    pe_t[:, ts(mc, MC)],
                                    start=True, stop=False,
                                    skip_group_check=True)
                            else:
                                nc.tensor.matmul(
                                    pav_t[ti][ds(strip, D), :], pvts,
                                    pe_t[:, ts(mc, MC)],
                                    start=False, stop=(pnt == NTILES - 1),
                                    tile_position=(0, strip),
                                    skip_group_check=True)
                        return go

                    chains = [mk(mc) for mc in range(8)]

                # weave: score chunks c0+c1 first (they gate the EXP chain),
                # then the prev step's AV block (gated by its vts), then c2.
                off = 0
                for ci, csz in enumerate(SCH):
                    s_ps = sp.tile([NT, max(SCH)], F32, tag="sa", name="s_ps")
                    for i in range(csz // MC):
                        b0 = 64 if (off // MC + i) % 2 else 0
                        nc.tensor.matmul(
                            s_ps[:, ts(i, MC)],
                            qsp[ds(b0, 3 * D), ds(h * N + nt * NT, NT)],
                            ksp[ds(b0, 3 * D), ds(h * N + off + i * MC, MC)],
                            start=True, stop=True)
                    nc.scalar.activation(
                        e_t[:, ds(off, csz)], s_ps[:, ds(0, csz)],
                        AF.Exp, accum_out=rsp[:, ds(ci, 1)])
                    off += csz
                    if ci == 1:
                        for _ in range(6):
                            if chains:
                                chains.pop(0)()
                while chains:
                    chains.pop(0)()
                if s < 16:
                    # v for n-tile s+16 via xbar transpose (serial on the
                    # sync engine, ~1.2us each -- one per step is free here)
                    nc.sync.dma_start(vT[:, ts(s + 16, 32)],
                                      hi_t[ds(64, 32), ts(s + 16, NT)],
                                      transpose=True)

                rs = rp.tile([NT, 1], F32, tag="r1", name="rs1")
                rinv = rp.tile([NT, 1], F32, tag="ri", name="rinv")
                nc.vector.reduce_sum(rs[:], rsp[:, ds(0, 3)],
                                     axis=mybir.AxisListType.X)
                nc.vector.reciprocal(rinv[:], rs[:])
                if nt == 0:
                    vts = av_state[h][1][:, ds(0, D)]
                else:
                    vts_t = vp.tile([NT, D], BF16, tag="vts",
                                    name=f"vts{h}_{nt}")
                    vts = vts_t[:]
                nc.vector.tensor_scalar_mul(
                    vts, vT[:, ds(nt * 32 + h * D, D)], rinv[:])
                prev = (h, nt, e_t, vts)

                if s == NTILES:
                    # AV(h0, 31) was just woven above; drain head-0 chains
                    emit_evac(0, "vector")

            # ---- tail: flush AV(h1, 31), drain, project ----
            ph, pnt, pe_t, pvts = prev
            pav_t, _ = av_state[ph]
            for mc in range(8):
                ti, strip = mc // 4, 32 * (mc % 4)
                nc.tensor.matmul(
                    pav_t[ti][ds(strip, D), :], pvts, pe_t[:, ts(mc, MC)],
                    start=False, stop=True, tile_position=(0, strip),
                    skip_group_check=True)
            emit_evac(1, "both")

            # y[:, mc] = sum_h wps_h^T out_h[:, mc]; wps replicas at every
            # 32-strip let lhsT/rhs share the strip's base partition.
            # yp rotates through 4 PSUM slots (sa x2 + the freed av banks)
            # so the MM -> copy -> DMA chain pipelines 4 deep.
            for mc in range(8):
                ti, strip = mc // 4, 32 * (mc % 4)
                tag = ("sa", "av0", "av1")[mc % 3]
                yp = sp.tile([C, MC], F32, tag="sa", name=f"yp{mc}") \
                    if tag == "sa" else \
                    avp.tile([C, MC], F32, tag=tag, name=f"yp{mc}")
                for hh in range(2):
                    nc.tensor.matmul(
                        yp[:], wps_t[hh][ds(strip, D), :],
                        out_sb[hh][ds(strip, D), ts(ti, MC)],
                        start=(hh == 0), stop=(hh == 1),
                        tile_position=(strip, 0))
                if mc % 2 == 0:
                    nc.vector.tensor_copy(y_sb[:, ts(mc, MC)], yp[:])
                else:
                    nc.scalar.copy(y_sb[:, ts(mc, MC)], yp[:])
                dq2 = nc.sync if mc % 2 == 0 else nc.scalar
                dq2.dma_start(y[:, ts(mc, MC)], y_sb[:, ts(mc, MC)])


_PROGRAM = None


def _get_program():
    global _PROGRAM
    if _PROGRAM is None:
        nc = bacc.Bacc("TRN2", target_bir_lowering=False, debug=False,
                       num_devices=8)
        t1 = nc.dram_tensor("t1", [C + 1, N], BF16, kind="ExternalInput").ap()
        t2 = nc.dram_tensor("t2", [2 * C, N], BF16, kind="ExternalInput").ap()
        whiB = nc.dram_tensor("whiB", [C + 1, 96], BF16, kind="ExternalInput").ap()
        whlB = nc.dram_tensor("whlB", [2 * C, 96], BF16, kind="ExternalInput").ap()
        wv1 = nc.dram_tensor("wv1", [C + 1, 32], BF16, kind="ExternalInput").ap()
        wv2 = nc.dram_tensor("wv2", [2 * C, 32], BF16, kind="ExternalInput").ap()
        wps0 = nc.dram_tensor("wps0", [112, C], BF16, kind="ExternalInput").ap()
        wps1 = nc.dram_tensor("wps1", [112, C], BF16, kind="ExternalInput").ap()
        y = nc.dram_tensor("y", [C, N], F32, kind="ExternalOutput").ap()
        with tile.TileContext(nc) as tc:
            _body(tc, y, t1, t2, whiB, whlB, wv1, wv2, wps0, wps1)
        nc.compile()
        _PROGRAM = nc
    return _PROGRAM


def _make_in_maps(x, qkv_w, qkv_b, proj_w, proj_b=None):
    x = np.asarray(x, dtype=np.float32)
    qkv_w = np.asarray(qkv_w, dtype=np.float32)
    qkv_b = np.asarray(qkv_b, dtype=np.float32)
    proj_w = np.asarray(proj_w, dtype=np.float32)

    # per-batch tensors (shared by the 2 cores of a batch)
    t1s, t2s = [], []
    for b in range(B):
        xf = x[b].reshape(C, N)
        xh = xf.astype(BF)
        xl = (xf - xh.astype(np.float32)).astype(BF)
        t1s.append(np.ascontiguousarray(
            np.concatenate([xh, np.ones((1, N), BF)], axis=0)))
        t2s.append(np.ascontiguousarray(np.concatenate([xl, xh], axis=0)))

    in_maps = []
    for core in range(8):
        b = core // 2
        h0 = 2 * (core % 2)
        heads = (h0, h0 + 1)

        # weight stack cols: [q_h0, q_h1, k_h0, k_h1, v_h0, v_h1]
        rows = []
        bias = []
        for blk in range(3):
            for h in heads:
                r = slice(blk * C + h * D, blk * C + (h + 1) * D)
                rows.append(qkv_w[r, :])
                bias.append(qkv_b[r])
        Wsel = np.concatenate(rows, axis=0)          # [96, 64]
        bsel = np.concatenate(bias, axis=0)          # [96]
        Whi = Wsel.astype(BF)
        Wlo = (Wsel - Whi.astype(np.float32)).astype(BF)
        whiB = np.zeros((C + 1, 96), BF)
        whiB[:C] = Whi.T
        whiB[C] = bsel.astype(BF)
        whlB = np.ascontiguousarray(
            np.concatenate([Whi.T.astype(BF), Wlo.T.astype(BF)], axis=0))
        # v weights for the early token-partition v matmuls (cols 64-95 of
        # the stack are the v heads)
        wv1B = np.zeros((C + 1, 32), BF)
        wv1B[:C] = Wlo.T[:, 64:96]
        wv1B[C] = bsel[64:96].astype(BF)
        wv2B = np.ascontiguousarray(
            np.concatenate([Whi.T[:, 64:96], Whi.T[:, 64:96]], axis=0))

        # proj weights, replicated at every 32-partition strip so the proj
        # matmul's lhsT base partition matches its rhs strip
        wpss = []
        for h in heads:
            w = np.zeros((112, C), BF)
            blk = proj_w[:, h * D:(h + 1) * D].T.astype(BF)
            for strip in range(4):
                w[strip * 32:strip * 32 + D, :] = blk
            wpss.append(w)

        in_maps.append({
            "t1": t1s[b],
            "t2": t2s[b],
            "whiB": whiB,
            "whlB": whlB,
            "wv1": wv1B,
            "wv2": wv2B,
            "wps0": wpss[0],
            "wps1": wpss[1],
        })
    return in_maps


def run_cores(inputs, **kw):
    """Compile+run on the 8 cores; returns BassKernelResults."""
    nc = _get_program()
    in_maps = _make_in_maps(**inputs)
    return run_bass_kernel_spmd(nc, in_maps, list(range(8)), **kw)


def kernel(x, qkv_w, qkv_b, proj_w, proj_b):
    res = run_cores(dict(x=x, qkv_w=qkv_w, qkv_b=qkv_b,
                         proj_w=proj_w, proj_b=proj_b))
    proj_b = np.asarray(proj_b, dtype=np.float32)
    parts = [np.asarray(r["y"], dtype=np.float32) for r in res.results]
    out = np.empty((B, C, N), np.float32)
    for b in range(B):
        out[b] = parts[2 * b] + parts[2 * b + 1] + proj_b[:, None]
    return out.reshape(B, C, 64, 64)


if __name__ == "__main__":
    _get_program()
    print("program built OK")


# revision 28
# speedup vs baseline: 1.0472x; 1.0079x over previous
"""Trainium2 Bass kernel for nn_Attention_82540681494971.

Spatial self-attention block (LDM AttnBlock style, unscaled):
  qkv = conv1x1(x);  s = q^T k  [n x n] per (b,head);  attn = softmax(s, axis=-1)
  out[d,m] = sum_n v[d,n] attn[n,m];  y = conv1x1(out)

Shapes: B=4, C=64, H=W=64 -> n=4096 tokens, HEAD=4, d=16.

Sharding: 8 cores, core c handles batch b=c//2 and heads (0,1) if c%2==0
else (2,3). Each core computes a partial projection output over its two
heads' channels; host sums the two partials per batch and adds proj bias.

Key algebra: attn[n,m] = E[n,m]/rowsum[n] with E=exp(s). Since the AV
contraction runs over n (the softmax row index), fold 1/rowsum into v:
  out[d,m] = sum_n (v[d,n]*rinv[n]) E[n,m]
so the big E matrix never needs normalizing.

Engine plan (ACT is the hard floor: 33.5M exp/core @ 1 elem/cyc/lane):
 - qkv: host pre-splits x and W into bf16 hi/lo; 2 bf16 matmuls per
   512-token chunk produce q,k,v for BOTH heads at once (M=96).
 - scores: exact-fp32-grade via a 3-term bf16 split (drop lo*lo):
     s = [q_hi;q_lo;q_hi]^T [k_hi;k_hi;k_lo]   (K=48 stacked)
 - AV: 8 persistent col-tiled accumulation chains per head live in 2
   PSUM banks for the whole head (tile_position cols 0/32/64/96); the
   first matmul of each bank is widened to M=112 with a zero-padded
   vts so the whole bank gets defined values + has_written bits.
 - exp: 3 chunked ACTIVATEs (1536/1536/1024) per 128-row block with
   accum_out giving the rowsum; ping-pong over 2x3 PSUM banks.
 - proj: outsb (bf16) stacked for both heads, single K=64 bf16 matmul.
"""

import numpy as np
import ml_dtypes
from contextlib import ExitStack

import concourse.bass as bass
import concourse.mybir as mybir
import concourse.tile as tile
from concourse import bacc
from concourse.bass import ts, ds
from concourse.bass_utils import run_bass_kernel_spmd

F32 = mybir.dt.float32
BF16 = mybir.dt.bfloat16
AF = mybir.ActivationFunctionType
BF = ml_dtypes.bfloat16

B, C, HEAD, D = 4, 64, 4, 16
N = 4096          # tokens = H*W
NT = 128          # n-tile (partition) size
NTILES = N // NT  # 32
MC = 512          # matmul free-dim chunk
SCH = (1024, 1536, 1536)  # scores/exp PSUM chunking (small chunk first so
                          # the first EXP needs only one assembled quarter)


def _body(tc, y, t1, t2, whiB, whlB, wv1, wv2, wps0, wps1):
    nc = tc.nc
    ctx = ExitStack()
    with ctx:
        pp = ctx.enter_context(tc.tile_pool(name="persist", bufs=1))
        cp = ctx.enter_context(tc.tile_pool(name="consts", bufs=1))

        # ---- persistent SBUF ----
        t1_t = pp.tile([C + 1, N], BF16)   # [x_hi; ones]
        t2_t = pp.tile([2 * C, N], BF16)   # [x_lo; x_hi]
        # score stacks mirrored at partition 64: consecutive score matmuls
        # alternate PE row groups so each LDWEIGHTS hides under the other
        # group's stream (without this every matmul pays ldw serially).
        qsp = pp.tile([64 + 3 * D, 2 * N], BF16)  # [q_hi; q_lo; q_hi] x2
        ksp = pp.tile([64 + 3 * D, 2 * N], BF16)  # [k_hi; k_hi; k_lo] x2
        hi_t = pp.tile([96, N], BF16)      # bf16 of qkv psum (q0 q1 k0 k1 v0 v1)
        lo_t = pp.tile([C, N], BF16)       # residual for q,k rows
        vT = pp.tile([NT, 32 * NTILES], BF16)  # per n-tile [128, 32] (v h0|h1)
        # AV results, one [112, 1024] buffer per head; av tile ti -> cols
        # ti*512, m-chunk = 4*ti + strip/32 on partition rows strip..strip+16
        out_sb = [pp.tile([112, 1024], BF16, tag=f"osb{h}", name=f"osb{h}")
                  for h in range(2)]
        y_sb = pp.tile([C, N], F32)
        wtile = pp.tile([NT, MC], BF16)    # PE warm-up fodder
        # zero-padded vts tiles (cols 16-111 stay 0 for the widened first
        # AV matmul); memset early so phase-1 gpsimd work isn't blocked
        # behind the gpsimd-dispatched assembly DMAs
        vpads = [pp.tile([NT, 112], BF16, name=f"vpad{h}") for h in range(2)]
        nc.gpsimd.memset(vpads[0][:], 0.0)
        nc.gpsimd.memset(vpads[1][:], 0.0)

        # ---- constants ----
        whi_t = cp.tile([C + 1, 96], BF16)
        whl_t = cp.tile([2 * C, 96], BF16)
        wv1_t = cp.tile([C + 1, 32], BF16)   # [wv_lo; bv]
        wv2_t = cp.tile([2 * C, 32], BF16)   # [wv_hi; wv_hi]
        wps_t = [cp.tile([112, C], BF16, name=f"wps{h}") for h in range(2)]
        nc.gpsimd.memset(wtile[:], 0.0)
        for c in range(8):
            nc.sync.dma_start(t1_t[:, ts(c, MC)], t1[:, ts(c, MC)])
            nc.sync.dma_start(t2_t[:, ts(c, MC)], t2[:, ts(c, MC)])
            if c == 0:
                nc.sync.dma_start(whi_t[:], whiB[:])
                nc.sync.dma_start(whl_t[:], whlB[:])
                nc.sync.dma_start(wv1_t[:], wv1[:])
                nc.sync.dma_start(wv2_t[:], wv2[:])
        nc.sync.dma_start(wps_t[0][:], wps0[:])
        nc.sync.dma_start(wps_t[1][:], wps1[:])

        # ---- phase 0: fused qkv + bf16 hi/lo split ----
        with tc.tile_pool(name="p0psum", bufs=2, space="PSUM") as p0:
            # warm the PE's HAM clock gate while DMAs land: ~5us of dummy
            # matmuls with no DMA dependency (wtile is memset on-chip)
            for w in range(12):
                wps = p0.tile([NT, MC], F32, tag="warm", bufs=2,
                              name=f"warm{w}")
                nc.tensor.matmul(wps[:], wtile[:, ds(0, NT)], wtile[:],
                                 start=True, stop=True)
            for g in range(4):  # 1024-wide groups: fewer, larger evac ops
                ps = p0.tile([96, 2 * MC], F32, tag="p0", name=f"qkv{g}")
                for ci in range(2):
                    c = 2 * g + ci
                    nc.tensor.matmul(ps[:, ts(ci, MC)], whi_t[:],
                                     t1_t[:, ts(c, MC)], start=True, stop=False)
                    nc.tensor.matmul(ps[:, ts(ci, MC)], whl_t[:],
                                     t2_t[:, ts(c, MC)], start=False, stop=True)
                # hi on ACT (idle in phase 0), lo on DVE
                nc.scalar.copy(hi_t[:, ts(g, 2 * MC)], ps[:])
                nc.vector.tensor_sub(lo_t[:, ts(g, 2 * MC)], ps[ds(0, C), :],
                                     hi_t[ds(0, C), ts(g, 2 * MC)])
            # v for n-tiles 0-7 on the PE (token-partition form); only these
            # are needed early, the rest transpose in phase 1
            for nt in range(8):
                psv = p0.tile([NT, 32], F32, tag="pv", bufs=2,
                              name=f"psv{nt}")
                nc.tensor.matmul(psv[:], t2_t[:, ds(nt * NT, NT)],
                                 wv2_t[:], start=True, stop=False)
                nc.tensor.matmul(psv[:], t1_t[:, ds(nt * NT, NT)],
                                 wv1_t[:], start=False, stop=True)
                nc.vector.tensor_copy(vT[:, ts(nt, 32)], psv[:])

            # assembly into the mirrored K=48 score stacks (SBUF->SBUF DMA;
            # engines can't write partition base 16). SBUF->SBUF DMA runs at
            # only ~110GB/s serially on the dispatching queue, so it can't be
            # made fast -- it must be HIDDEN: emit in dependency-priority
            # order (k h0 quarter 0 first, unblocking the first EXPs) and
            # let the rest stream in behind the early attention steps.
            def asm(eng, t, h, qt):
                dsl = ds(h * N + qt * 1024, 1024)
                sl = ds(qt * 1024, 1024)
                if t == "k":
                    row = 32 + h * D
                    blocks = ((0, hi_t), (D, hi_t), (2 * D, lo_t))
                else:
                    row = h * D
                    blocks = ((0, hi_t), (D, lo_t), (2 * D, hi_t))
                dst = ksp if t == "k" else qsp
                for boff, src in blocks:
                    eng.dma_start(dst[ds(boff, D), dsl], src[ds(row, D), sl])
                # mirror the whole 48-row stack to partition 64 in one op
                eng.dma_start(dst[ds(64, 3 * D), dsl], dst[ds(0, 3 * D), dsl])

            # head 0 on the sync queue in priority order (the sequencer
            # costs ~0.6us per DMA serially); all of head 1 on the gpsimd
            # software DGE, whose dispatch overlaps sync's
            asm(nc.sync, "k", 0, 0)
            asm(nc.sync, "q", 0, 0)
            for qt in (1, 2, 3):
                asm(nc.sync, "k", 0, qt)
            for qt in (1, 2, 3):
                asm(nc.sync, "q", 0, qt)
            # v transposes for n-tiles 8-15 (rest are woven into phase 1)
            for nt in range(8, 16):
                nc.sync.dma_start(vT[:, ts(nt, 32)],
                                  hi_t[ds(64, 32), ts(nt, NT)],
                                  transpose=True)
            for qt in range(4):
                asm(nc.gpsimd, "k", 1, qt)
                asm(nc.gpsimd, "q", 1, qt)

        # ---- phase 1: attention, software-pipelined ----
        # Per step (h, nt): emit this n-tile's score matmuls + exp, woven
        # with the AV matmuls of the previous step. AV chains accumulate in
        # PSUM across the whole head (col-tiled 4-way, 2 banks per head).
        with (
            tc.tile_pool(name="ep", bufs=3) as ep,
            tc.tile_pool(name="rp", bufs=4) as rp,
            tc.tile_pool(name="vp", bufs=3) as vp,
            tc.tile_pool(name="sapsum", bufs=2, space="PSUM") as sp,
            tc.tile_pool(name="avpsum", bufs=1, space="PSUM") as avp,
        ):
            av_state = {}

            def emit_evac(h, engines):
                av_t, _ = av_state[h]
                for ti in range(2):
                    dst = out_sb[h][:, ts(ti, MC)]
                    if engines == "both" and ti == 1:
                        nc.scalar.copy(dst, av_t[ti][:])
                    else:
                        nc.vector.tensor_copy(dst, av_t[ti][:])

            prev = None
            for s in range(64):
                h, nt = divmod(s, NTILES)
                if nt == 0:
                    av_t = [avp.tile([112, MC], F32, tag=f"av{i}",
                                     name=f"av{i}h{h}") for i in range(2)]
                    av_state[h] = (av_t, vpads[h])

                e_t = ep.tile([NT, N], BF16, tag="e", name=f"e{h}_{nt}")
                rsp = rp.tile([NT, 4], F32, tag="rs", name="rsp")

                chains = []
                if prev is not None:
                    ph, pnt, pe_t, pvts = prev
                    pav_t, pvpad = av_state[ph]

                    def mk(mc, pnt=pnt, pe_t=pe_t, pvts=pvts,
                           pav_t=pav_t, pvpad=pvpad):
                        def go():
                            ti, strip = mc // 4, 32 * (mc % 4)
                            if pnt == 0 and mc % 4 == 0:
                                # widened first matmul: writes the vts
                                # product on partitions 0-15 and zeros on
                                # 16-111, claiming the whole bank.
                                nc.tensor.matmul(
                                    pav_t[ti][:, :], pvpad[:, ds(0, 112)],
                                    pe_t[:, ts(mc, MC)],
                                    start=True, stop=False,
                                    skip_group_check=True)
                            else:
                                nc.tensor.matmul(
                                    pav_t[ti][ds(strip, D), :], pvts,
                                    pe_t[:, ts(mc, MC)],
                                    start=False, stop=(pnt == NTILES - 1),
                                    tile_position=(0, strip),
                                    skip_group_check=True)
                        return go

                    chains = [mk(mc) for mc in range(8)]

                # weave: score chunks c0+c1 first (they gate the EXP chain),
                # then the prev step's AV block (gated by its vts), then c2.
                off = 0
                for ci, csz in enumerate(SCH):
                    s_ps = sp.tile([NT, max(SCH)], F32, tag="sa", name="s_ps")
                    for i in range(csz // MC):
                        b0 = 64 if (off // MC + i) % 2 else 0
                        nc.tensor.matmul(
                            s_ps[:, ts(i, MC)],
                            qsp[ds(b0, 3 * D), ds(h * N + nt * NT, NT)],
                            ksp[ds(b0, 3 * D), ds(h * N + off + i * MC, MC)],
                            start=True, stop=True)
                    nc.scalar.activation(
                        e_t[:, ds(off, csz)], s_ps[:, ds(0, csz)],
                        AF.Exp, accum_out=rsp[:, ds(ci, 1)])
                    off += csz
                    if ci == 1:
                        for _ in range(6):
                            if chains:
                                chains.pop(0)()
                while chains:
                    chains.pop(0)()
                if s < 16:
                    # v for n-tile s+16 via xbar transpose (serial on the
                    # sync engine, ~1.2us each -- one per step is free here)
                    nc.sync.dma_start(vT[:, ts(s + 16, 32)],
                                      hi_t[ds(64, 32), ts(s + 16, NT)],
                                      transpose=True)

                rs = rp.tile([NT, 1], F32, tag="r1", name="rs1")
                rinv = rp.tile([NT, 1], F32, tag="ri", name="rinv")
                nc.vector.reduce_sum(rs[:], rsp[:, ds(0, 3)],
                                     axis=mybir.AxisListType.X)
                nc.vector.reciprocal(rinv[:], rs[:])
                if nt == 0:
                    vts = av_state[h][1][:, ds(0, D)]
                else:
                    vts_t = vp.tile([NT, D], BF16, tag="vts",
                                    name=f"vts{h}_{nt}")
                    vts = vts_t[:]
                nc.vector.tensor_scalar_mul(
                    vts, vT[:, ds(nt * 32 + h * D, D)], rinv[:])
                prev = (h, nt, e_t, vts)

                if s == NTILES:
                    # AV(h0, 31) was just woven above; drain head-0 chains
                    emit_evac(0, "vector")

            # ---- tail: flush AV(h1, 31), drain, project ----
            ph, pnt, pe_t, pvts = prev
            pav_t, _ = av_state[ph]
            for mc in range(8):
                ti, strip = mc // 4, 32 * (mc % 4)
                nc.tensor.matmul(
                    pav_t[ti][ds(strip, D), :], pvts, pe_t[:, ts(mc, MC)],
                    start=False, stop=True, tile_position=(0, strip),
                    skip_group_check=True)
            emit_evac(1, "both")

            # y[:, mc] = sum_h wps_h^T out_h[:, mc]; wps replicas at every
            # 32-strip let lhsT/rhs share the strip's base partition.
            # yp rotates through 4 PSUM slots (sa x2 + the freed av banks)
            # so the MM -> copy -> DMA chain pipelines 4 deep.
            for mc in range(8):
                ti, strip = mc // 4, 32 * (mc % 4)
                tag = ("sa", "av0", "av1")[mc % 3]
                yp = sp.tile([C, MC], F32, tag="sa", name=f"yp{mc}") \
                    if tag == "sa" else \
                    avp.tile([C, MC], F32, tag=tag, name=f"yp{mc}")
                for hh in range(2):
                    nc.tensor.matmul(
                        yp[:], wps_t[hh][ds(strip, D), :],
                        out_sb[hh][ds(strip, D), ts(ti, MC)],
                        start=(hh == 0), stop=(hh == 1),
                        tile_position=(strip, 0))
                if mc % 2 == 0:
                    nc.vector.tensor_copy(y_sb[:, ts(mc, MC)], yp[:])
                else:
                    nc.scalar.copy(y_sb[:, ts(mc, MC)], yp[:])
                dq2 = nc.sync if mc % 2 == 0 else nc.scalar
                dq2.dma_start(y[:, ts(mc, MC)], y_sb[:, ts(mc, MC)])


_PROGRAM = None


def _get_program():
    global _PROGRAM
    if _PROGRAM is None:
        nc = bacc.Bacc("TRN2", target_bir_lowering=False, debug=False,
                       num_devices=8)
        t1 = nc.dram_tensor("t1", [C + 1, N], BF16, kind="ExternalInput").ap()
        t2 = nc.dram_tensor("t2", [2 * C, N], BF16, kind="ExternalInput").ap()
        whiB = nc.dram_tensor("whiB", [C + 1, 96], BF16, kind="ExternalInput").ap()
        whlB = nc.dram_tensor("whlB", [2 * C, 96], BF16, kind="ExternalInput").ap()
        wv1 = nc.dram_tensor("wv1", [C + 1, 32], BF16, kind="ExternalInput").ap()
        wv2 = nc.dram_tensor("wv2", [2 * C, 32], BF16, kind="ExternalInput").ap()
        wps0 = nc.dram_tensor("wps0", [112, C], BF16, kind="ExternalInput").ap()
        wps1 = nc.dram_tensor("wps1", [112, C], BF16, kind="ExternalInput").ap()
        y = nc.dram_tensor("y", [C, N], F32, kind="ExternalOutput").ap()
        with tile.TileContext(nc) as tc:
            _body(tc, y, t1, t2, whiB, whlB, wv1, wv2, wps0, wps1)
        nc.compile()
        _PROGRAM = nc
    return _PROGRAM


def _make_in_maps(x, qkv_w, qkv_b, proj_w, proj_b=None):
    x = np.asarray(x, dtype=np.float32)
    qkv_w = np.asarray(qkv_w, dtype=np.float32)
    qkv_b = np.asarray(qkv_b, dtype=np.float32)
    proj_w = np.asarray(proj_w, dtype=np.float32)

    # per-batch tensors (shared by the 2 cores of a batch)
    t1s, t2s = [], []
    for b in range(B):
        xf = x[b].reshape(C, N)
        xh = xf.astype(BF)
        xl = (xf - xh.astype(np.float32)).astype(BF)
        t1s.append(np.ascontiguousarray(
            np.concatenate([xh, np.ones((1, N), BF)], axis=0)))
        t2s.append(np.ascontiguousarray(np.concatenate([xl, xh], axis=0)))

    in_maps = []
    for core in range(8):
        b = core // 2
        h0 = 2 * (core % 2)
        heads = (h0, h0 + 1)

        # weight stack cols: [q_h0, q_h1, k_h0, k_h1, v_h0, v_h1]
        rows = []
        bias = []
        for blk in range(3):
            for h in heads:
                r = slice(blk * C + h * D, blk * C + (h + 1) * D)
                rows.append(qkv_w[r, :])
                bias.append(qkv_b[r])
        Wsel = np.concatenate(rows, axis=0)          # [96, 64]
        bsel = np.concatenate(bias, axis=0)          # [96]
        Whi = Wsel.astype(BF)
        Wlo = (Wsel - Whi.astype(np.float32)).astype(BF)
        whiB = np.zeros((C + 1, 96), BF)
        whiB[:C] = Whi.T
        whiB[C] = bsel.astype(BF)
        whlB = np.ascontiguousarray(
            np.concatenate([Whi.T.astype(BF), Wlo.T.astype(BF)], axis=0))
        # v weights for the early token-partition v matmuls (cols 64-95 of
        # the stack are the v heads)
        wv1B = np.zeros((C + 1, 32), BF)
        wv1B[:C] = Wlo.T[:, 64:96]
        wv1B[C] = bsel[64:96].astype(BF)
        wv2B = np.ascontiguousarray(
            np.concatenate([Whi.T[:, 64:96], Whi.T[:, 64:96]], axis=0))

        # proj weights, replicated at every 32-partition strip so the proj
        # matmul's lhsT base partition matches its rhs strip
        wpss = []
        for h in heads:
            w = np.zeros((112, C), BF)
            blk = proj_w[:, h * D:(h + 1) * D].T.astype(BF)
            for strip in range(4):
                w[strip * 32:strip * 32 + D, :] = blk
            wpss.append(w)

        in_maps.append({
            "t1": t1s[b],
            "t2": t2s[b],
            "whiB": whiB,
            "whlB": whlB,
            "wv1": wv1B,
            "wv2": wv2B,
            "wps0": wpss[0],
            "wps1": wpss[1],
        })
    return in_maps


def run_cores(inputs, **kw):
    """Compile+run on the 8 cores; returns BassKernelResults."""
    nc = _get_program()
    in_maps = _make_in_maps(**inputs)
    return run_bass_kernel_spmd(nc, in_maps, list(range(8)), **kw)


def kernel(x, qkv_w, qkv_b, proj_w, proj_b):
    res = run_cores(dict(x=x, qkv_w=qkv_w, qkv_b=qkv_b,
                         proj_w=proj_w, proj_b=proj_b))
    proj_b = np.asarray(proj_b, dtype=np.float32)
    parts = [np.asarray(r["y"], dtype=np.float32) for r in res.results]
    out = np.empty((B, C, N), np.float32)
    for b in range(B):
        out[b] = parts[2 * b] + parts[2 * b + 1] + proj_b[:, None]
    return out.reshape(B, C, 64, 64)


if __name__ == "__main__":
    _get_program()
    print("program built OK")


# revision 32
# speedup vs baseline: 1.0483x; 1.0011x over previous
"""Trainium2 Bass kernel for nn_Attention_82540681494971.

Spatial self-attention block (LDM AttnBlock style, unscaled):
  qkv = conv1x1(x);  s = q^T k  [n x n] per (b,head);  attn = softmax(s, axis=-1)
  out[d,m] = sum_n v[d,n] attn[n,m];  y = conv1x1(out)

Shapes: B=4, C=64, H=W=64 -> n=4096 tokens, HEAD=4, d=16.

Sharding: 8 cores, core c handles batch b=c//2 and heads (0,1) if c%2==0
else (2,3). Each core computes a partial projection output over its two
heads' channels; host sums the two partials per batch and adds proj bias.

Key algebra: attn[n,m] = E[n,m]/rowsum[n] with E=exp(s). Since the AV
contraction runs over n (the softmax row index), fold 1/rowsum into v:
  out[d,m] = sum_n (v[d,n]*rinv[n]) E[n,m]
so the big E matrix never needs normalizing.

Engine plan (ACT is the hard floor: 33.5M exp/core @ 1 elem/cyc/lane):
 - qkv: host pre-splits x and W into bf16 hi/lo; 2 bf16 matmuls per
   512-token chunk produce q,k,v for BOTH heads at once (M=96).
 - scores: exact-fp32-grade via a 3-term bf16 split (drop lo*lo):
     s = [q_hi;q_lo;q_hi]^T [k_hi;k_hi;k_lo]   (K=48 stacked)
 - AV: 8 persistent col-tiled accumulation chains per head live in 2
   PSUM banks for the whole head (tile_position cols 0/32/64/96); the
   first matmul of each bank is widened to M=112 with a zero-padded
   vts so the whole bank gets defined values + has_written bits.
 - exp: 3 chunked ACTIVATEs (1536/1536/1024) per 128-row block with
   accum_out giving the rowsum; ping-pong over 2x3 PSUM banks.
 - proj: outsb (bf16) stacked for both heads, single K=64 bf16 matmul.
"""

import numpy as np
import ml_dtypes
from contextlib import ExitStack

import concourse.bass as bass
import concourse.mybir as mybir
import concourse.tile as tile
from concourse import bacc
from concourse.bass import ts, ds
from concourse.bass_utils import run_bass_kernel_spmd

F32 = mybir.dt.float32
BF16 = mybir.dt.bfloat16
AF = mybir.ActivationFunctionType
BF = ml_dtypes.bfloat16

B, C, HEAD, D = 4, 64, 4, 16
N = 4096          # tokens = H*W
NT = 128          # n-tile (partition) size
NTILES = N // NT  # 32
MC = 512          # matmul free-dim chunk
SCH = (1024, 1536, 1536)  # scores/exp PSUM chunking (small chunk first so
                          # the first EXP needs only one assembled quarter)


def _body(tc, y, t1, t2, whiB, whlB, wv1, wv2, wps0, wps1):
    nc = tc.nc
    ctx = ExitStack()
    with ctx:
        pp = ctx.enter_context(tc.tile_pool(name="persist", bufs=1))
        cp = ctx.enter_context(tc.tile_pool(name="consts", bufs=1))

        # ---- persistent SBUF ----
        t1_t = pp.tile([C + 1, N], BF16)   # [x_hi; ones]
        t2_t = pp.tile([2 * C, N], BF16)   # [x_lo; x_hi]
        # score stacks mirrored at partition 64: consecutive score matmuls
        # alternate PE row groups so each LDWEIGHTS hides under the other
        # group's stream (without this every matmul pays ldw serially).
        qsp = pp.tile([64 + 3 * D, 2 * N], BF16)  # [q_hi; q_lo; q_hi] x2
        ksp = pp.tile([64 + 3 * D, 2 * N], BF16)  # [k_hi; k_hi; k_lo] x2
        hi_t = pp.tile([96, N], BF16)      # bf16 of qkv psum (q0 q1 k0 k1 v0 v1)
        lo_t = pp.tile([C, N], BF16)       # residual for q,k rows
        vT = pp.tile([NT, 32 * NTILES], BF16)  # per n-tile [128, 32] (v h0|h1)
        # AV results, one [112, 1024] buffer per head; av tile ti -> cols
        # ti*512, m-chunk = 4*ti + strip/32 on partition rows strip..strip+16
        out_sb = [pp.tile([112, 1024], BF16, tag=f"osb{h}", name=f"osb{h}")
                  for h in range(2)]
        y_sb = pp.tile([C, N], F32)
        wtile = pp.tile([NT, MC], BF16)    # PE warm-up fodder
        # zero-padded vts tiles (cols 16-111 stay 0 for the widened first
        # AV matmul); memset early so phase-1 gpsimd work isn't blocked
        # behind the gpsimd-dispatched assembly DMAs
        vpads = [pp.tile([NT, 112], BF16, name=f"vpad{h}") for h in range(2)]
        nc.gpsimd.memset(vpads[0][:], 0.0)
        nc.gpsimd.memset(vpads[1][:], 0.0)

        # ---- constants ----
        whi_t = cp.tile([C + 1, 96], BF16)
        whl_t = cp.tile([2 * C, 96], BF16)
        wv1_t = cp.tile([C + 1, 32], BF16)   # [wv_lo; bv]
        wv2_t = cp.tile([2 * C, 32], BF16)   # [wv_hi; wv_hi]
        wps_t = [cp.tile([112, C], BF16, name=f"wps{h}") for h in range(2)]
        nc.gpsimd.memset(wtile[:], 0.0)
        for c in range(4):
            nc.sync.dma_start(t1_t[:, ts(c, 2 * MC)], t1[:, ts(c, 2 * MC)])
            nc.sync.dma_start(t2_t[:, ts(c, 2 * MC)], t2[:, ts(c, 2 * MC)])
            if c == 0:
                nc.sync.dma_start(whi_t[:], whiB[:])
                nc.sync.dma_start(whl_t[:], whlB[:])
                nc.sync.dma_start(wv1_t[:], wv1[:])
                nc.sync.dma_start(wv2_t[:], wv2[:])
        nc.gpsimd.dma_start(wps_t[0][:], wps0[:])
        nc.gpsimd.dma_start(wps_t[1][:], wps1[:])

        # ---- phase 0: fused qkv + bf16 hi/lo split ----
        with tc.tile_pool(name="p0psum", bufs=2, space="PSUM") as p0:
            # warm the PE's HAM clock gate while DMAs land: ~5us of dummy
            # matmuls with no DMA dependency (wtile is memset on-chip)
            for w in range(8):
                wps = p0.tile([NT, MC], F32, tag="warm", bufs=2,
                              name=f"warm{w}")
                nc.tensor.matmul(wps[:], wtile[:, ds(0, NT)], wtile[:],
                                 start=True, stop=True)
            for g in range(4):  # 1024-wide groups: fewer, larger evac ops
                ps = p0.tile([96, 2 * MC], F32, tag="p0", name=f"qkv{g}")
                for ci in range(2):
                    c = 2 * g + ci
                    nc.tensor.matmul(ps[:, ts(ci, MC)], whi_t[:],
                                     t1_t[:, ts(c, MC)], start=True, stop=False)
                    nc.tensor.matmul(ps[:, ts(ci, MC)], whl_t[:],
                                     t2_t[:, ts(c, MC)], start=False, stop=True)
                # hi on ACT (idle in phase 0), lo on DVE
                nc.scalar.copy(hi_t[:, ts(g, 2 * MC)], ps[:])
                nc.vector.tensor_sub(lo_t[:, ts(g, 2 * MC)], ps[ds(0, C), :],
                                     hi_t[ds(0, C), ts(g, 2 * MC)])
            # v for n-tiles 0-7 on the PE (token-partition form); only these
            # are needed early, the rest transpose in phase 1
            for nt in range(8):
                psv = p0.tile([NT, 32], F32, tag="pv", bufs=2,
                              name=f"psv{nt}")
                nc.tensor.matmul(psv[:], t2_t[:, ds(nt * NT, NT)],
                                 wv2_t[:], start=True, stop=False)
                nc.tensor.matmul(psv[:], t1_t[:, ds(nt * NT, NT)],
                                 wv1_t[:], start=False, stop=True)
                nc.vector.tensor_copy(vT[:, ts(nt, 32)], psv[:])

            # assembly into the mirrored K=48 score stacks (SBUF->SBUF DMA;
            # engines can't write partition base 16). SBUF->SBUF DMA runs at
            # only ~110GB/s serially on the dispatching queue, so it can't be
            # made fast -- it must be HIDDEN: emit in dependency-priority
            # order (k h0 quarter 0 first, unblocking the first EXPs) and
            # let the rest stream in behind the early attention steps.
            def asm(eng, t, h, hf):
                dsl = ds(h * N + hf * 2048, 2048)
                sl = ds(hf * 2048, 2048)
                if t == "k":
                    row = 32 + h * D
                    blocks = ((0, hi_t), (D, hi_t), (2 * D, lo_t))
                else:
                    row = h * D
                    blocks = ((0, hi_t), (D, lo_t), (2 * D, hi_t))
                dst = ksp if t == "k" else qsp
                for boff, src in blocks:
                    eng.dma_start(dst[ds(boff, D), dsl], src[ds(row, D), sl])
                # mirror the whole 48-row stack to partition 64 in one op
                eng.dma_start(dst[ds(64, 3 * D), dsl], dst[ds(0, 3 * D), dsl])

            # head 0 on the sync queue in priority order (the sequencer
            # costs ~0.6us per DMA serially); all of head 1 on the gpsimd
            # software DGE, whose dispatch overlaps sync's
            asm(nc.sync, "k", 0, 0)
            asm(nc.sync, "q", 0, 0)
            asm(nc.sync, "k", 0, 1)
            asm(nc.sync, "q", 0, 1)
            # v transposes for n-tiles 8-15 (rest are woven into phase 1)
            for nt in range(8, 16):
                nc.sync.dma_start(vT[:, ts(nt, 32)],
                                  hi_t[ds(64, 32), ts(nt, NT)],
                                  transpose=True)
            for hf in range(2):
                asm(nc.gpsimd, "k", 1, hf)
                asm(nc.gpsimd, "q", 1, hf)

        # ---- phase 1: attention, software-pipelined ----
        # Per step (h, nt): emit this n-tile's score matmuls + exp, woven
        # with the AV matmuls of the previous step. AV chains accumulate in
        # PSUM across the whole head (col-tiled 4-way, 2 banks per head).
        with (
            tc.tile_pool(name="ep", bufs=3) as ep,
            tc.tile_pool(name="rp", bufs=4) as rp,
            tc.tile_pool(name="vp", bufs=3) as vp,
            tc.tile_pool(name="sapsum", bufs=2, space="PSUM") as sp,
            tc.tile_pool(name="avpsum", bufs=1, space="PSUM") as avp,
        ):
            av_state = {}

            def emit_evac(h, engines):
                av_t, _ = av_state[h]
                for ti in range(2):
                    dst = out_sb[h][:, ts(ti, MC)]
                    if engines == "both" and ti == 1:
                        nc.scalar.copy(dst, av_t[ti][:])
                    else:
                        nc.vector.tensor_copy(dst, av_t[ti][:])

            prev = None
            for s in range(64):
                h, nt = divmod(s, NTILES)
                if nt == 0:
                    av_t = [avp.tile([112, MC], F32, tag=f"av{i}",
                                     name=f"av{i}h{h}") for i in range(2)]
                    av_state[h] = (av_t, vpads[h])

                e_t = ep.tile([NT, N], BF16, tag="e", name=f"e{h}_{nt}")
                rsp = rp.tile([NT, 4], F32, tag="rs", name="rsp")

                chains = []
                if prev is not None:
                    ph, pnt, pe_t, pvts = prev
                    pav_t, pvpad = av_state[ph]

                    def mk(mc, pnt=pnt, pe_t=pe_t, pvts=pvts,
                           pav_t=pav_t, pvpad=pvpad):
                        def go():
                            ti, strip = mc // 4, 32 * (mc % 4)
                            if pnt == 0 and mc % 4 == 0:
                                # widened first matmul: writes the vts
                                # product on partitions 0-15 and zeros on
                                # 16-111, claiming the whole bank.
                                nc.tensor.matmul(
                                    pav_t[ti][:, :], pvpad[:, ds(0, 112)],
                                    pe_t[:, ts(mc, MC)],
                                    start=True, stop=False,
                                    skip_group_check=True)
                            else:
                                nc.tensor.matmul(
                                    pav_t[ti][ds(strip, D), :], pvts,
                                    pe_t[:, ts(mc, MC)],
                                    start=False, stop=(pnt == NTILES - 1),
                                    tile_position=(0, strip),
                                    skip_group_check=True)
                        return go

                    chains = [mk(mc) for mc in range(8)]

                # weave: score chunks c0+c1 first (they gate the EXP chain),
                # then the prev step's AV block (gated by its vts), then c2.
                off = 0
                for ci, csz in enumerate(SCH):
                    s_ps = sp.tile([NT, max(SCH)], F32, tag="sa", name="s_ps")
                    for i in range(csz // MC):
                        b0 = 64 if (off // MC + i) % 2 else 0
                        nc.tensor.matmul(
                            s_ps[:, ts(i, MC)],
                            qsp[ds(b0, 3 * D), ds(h * N + nt * NT, NT)],
                            ksp[ds(b0, 3 * D), ds(h * N + off + i * MC, MC)],
                            start=True, stop=True)
                    nc.scalar.activation(
                        e_t[:, ds(off, csz)], s_ps[:, ds(0, csz)],
                        AF.Exp, accum_out=rsp[:, ds(ci, 1)])
                    off += csz
                    if ci == 1:
                        for _ in range(6):
                            if chains:
                                chains.pop(0)()
                while chains:
                    chains.pop(0)()
                if s < 16:
                    # v for n-tile s+16 via xbar transpose (serial on the
                    # sync engine, ~1.2us each -- one per step is free here)
                    nc.sync.dma_start(vT[:, ts(s + 16, 32)],
                                      hi_t[ds(64, 32), ts(s + 16, NT)],
                                      transpose=True)

                rs = rp.tile([NT, 1], F32, tag="r1", name="rs1")
                rinv = rp.tile([NT, 1], F32, tag="ri", name="rinv")
                nc.vector.reduce_sum(rs[:], rsp[:, ds(0, 3)],
                                     axis=mybir.AxisListType.X)
                nc.vector.reciprocal(rinv[:], rs[:])
                if nt == 0:
                    vts = av_state[h][1][:, ds(0, D)]
                else:
                    vts_t = vp.tile([NT, D], BF16, tag="vts",
                                    name=f"vts{h}_{nt}")
                    vts = vts_t[:]
                nc.vector.tensor_scalar_mul(
                    vts, vT[:, ds(nt * 32 + h * D, D)], rinv[:])
                prev = (h, nt, e_t, vts)

                if s == NTILES:
                    # AV(h0, 31) was just woven above; drain head-0 chains
                    emit_evac(0, "vector")

            # ---- tail: flush AV(h1, 31), drain, project ----
            ph, pnt, pe_t, pvts = prev
            pav_t, _ = av_state[ph]
            for mc in range(8):
                ti, strip = mc // 4, 32 * (mc % 4)
                nc.tensor.matmul(
                    pav_t[ti][ds(strip, D), :], pvts, pe_t[:, ts(mc, MC)],
                    start=False, stop=True, tile_position=(0, strip),
                    skip_group_check=True)
            emit_evac(1, "both")

            # y[:, mc] = sum_h wps_h^T out_h[:, mc]; wps replicas at every
            # 32-strip let lhsT/rhs share the strip's base partition.
            # yp rotates through 4 PSUM slots (sa x2 + the freed av banks)
            # so the MM -> copy -> DMA chain pipelines 4 deep.
            for mc in range(8):
                ti, strip = mc // 4, 32 * (mc % 4)
                tag = ("sa", "av0", "av1")[mc % 3]
                yp = sp.tile([C, MC], F32, tag="sa", name=f"yp{mc}") \
                    if tag == "sa" else \
                    avp.tile([C, MC], F32, tag=tag, name=f"yp{mc}")
                for hh in range(2):
                    nc.tensor.matmul(
                        yp[:], wps_t[hh][ds(strip, D), :],
                        out_sb[hh][ds(strip, D), ts(ti, MC)],
                        start=(hh == 0), stop=(hh == 1),
                        tile_position=(strip, 0))
                if mc % 2 == 0:
                    nc.vector.tensor_copy(y_sb[:, ts(mc, MC)], yp[:])
                else:
                    nc.scalar.copy(y_sb[:, ts(mc, MC)], yp[:])
                nc.sync.dma_start(y[:, ts(mc, MC)], y_sb[:, ts(mc, MC)])


_PROGRAM = None


def _get_program():
    global _PROGRAM
    if _PROGRAM is None:
        nc = bacc.Bacc("TRN2", target_bir_lowering=False, debug=False,
                       num_devices=8)
        t1 = nc.dram_tensor("t1", [C + 1, N], BF16, kind="ExternalInput").ap()
        t2 = nc.dram_tensor("t2", [2 * C, N], BF16, kind="ExternalInput").ap()
        whiB = nc.dram_tensor("whiB", [C + 1, 96], BF16, kind="ExternalInput").ap()
        whlB = nc.dram_tensor("whlB", [2 * C, 96], BF16, kind="ExternalInput").ap()
        wv1 = nc.dram_tensor("wv1", [C + 1, 32], BF16, kind="ExternalInput").ap()
        wv2 = nc.dram_tensor("wv2", [2 * C, 32], BF16, kind="ExternalInput").ap()
        wps0 = nc.dram_tensor("wps0", [112, C], BF16, kind="ExternalInput").ap()
        wps1 = nc.dram_tensor("wps1", [112, C], BF16, kind="ExternalInput").ap()
        y = nc.dram_tensor("y", [C, N], F32, kind="ExternalOutput").ap()
        with tile.TileContext(nc) as tc:
            _body(tc, y, t1, t2, whiB, whlB, wv1, wv2, wps0, wps1)
        nc.compile()
        _PROGRAM = nc
    return _PROGRAM


def _make_in_maps(x, qkv_w, qkv_b, proj_w, proj_b=None):
    x = np.asarray(x, dtype=np.float32)
    qkv_w = np.asarray(qkv_w, dtype=np.float32)
    qkv_b = np.asarray(qkv_b, dtype=np.float32)
    proj_w = np.asarray(proj_w, dtype=np.float32)

    # per-batch tensors (shared by the 2 cores of a batch)
    t1s, t2s = [], []
    for b in range(B):
        xf = x[b].reshape(C, N)
        xh = xf.astype(BF)
        xl = (xf - xh.astype(np.float32)).astype(BF)
        t1s.append(np.ascontiguousarray(
            np.concatenate([xh, np.ones((1, N), BF)], axis=0)))
        t2s.append(np.ascontiguousarray(np.concatenate([xl, xh], axis=0)))

    in_maps = []
    for core in range(8):
        b = core // 2
        h0 = 2 * (core % 2)
        heads = (h0, h0 + 1)

        # weight stack cols: [q_h0, q_h1, k_h0, k_h1, v_h0, v_h1]
        rows = []
        bias = []
        for blk in range(3):
            for h in heads:
                r = slice(blk * C + h * D, blk * C + (h + 1) * D)
                rows.append(qkv_w[r, :])
                bias.append(qkv_b[r])
        Wsel = np.concatenate(rows, axis=0)          # [96, 64]
        bsel = np.concatenate(bias, axis=0)          # [96]
        Whi = Wsel.astype(BF)
        Wlo = (Wsel - Whi.astype(np.float32)).astype(BF)
        whiB = np.zeros((C + 1, 96), BF)
        whiB[:C] = Whi.T
        whiB[C] = bsel.astype(BF)
        whlB = np.ascontiguousarray(
            np.concatenate([Whi.T.astype(BF), Wlo.T.astype(BF)], axis=0))
        # v weights for the early token-partition v matmuls (cols 64-95 of
        # the stack are the v heads)
        wv1B = np.zeros((C + 1, 32), BF)
        wv1B[:C] = Wlo.T[:, 64:96]
        wv1B[C] = bsel[64:96].astype(BF)
        wv2B = np.ascontiguousarray(
            np.concatenate([Whi.T[:, 64:96], Whi.T[:, 64:96]], axis=0))

        # proj weights, replicated at every 32-partition strip so the proj
        # matmul's lhsT base partition matches its rhs strip
        wpss = []
        for h in heads:
            w = np.zeros((112, C), BF)
            blk = proj_w[:, h * D:(h + 1) * D].T.astype(BF)
            for strip in range(4):
                w[strip * 32:strip * 32 + D, :] = blk
            wpss.append(w)

        in_maps.append({
            "t1": t1s[b],
            "t2": t2s[b],
            "whiB": whiB,
            "whlB": whlB,
            "wv1": wv1B,
            "wv2": wv2B,
            "wps0": wpss[0],
            "wps1": wpss[1],
        })
    return in_maps


def run_cores(inputs, **kw):
    """Compile+run on the 8 cores; returns BassKernelResults."""
    nc = _get_program()
    in_maps = _make_in_maps(**inputs)
    return run_bass_kernel_spmd(nc, in_maps, list(range(8)), **kw)


def kernel(x, qkv_w, qkv_b, proj_w, proj_b):
    res = run_cores(dict(x=x, qkv_w=qkv_w, qkv_b=qkv_b,
                         proj_w=proj_w, proj_b=proj_b))
    proj_b = np.asarray(proj_b, dtype=np.float32)
    parts = [np.asarray(r["y"], dtype=np.float32) for r in res.results]
    out = np.empty((B, C, N), np.float32)
    for b in range(B):
        out[b] = parts[2 * b] + parts[2 * b + 1] + proj_b[:, None]
    return out.reshape(B, C, 64, 64)


if __name__ == "__main__":
    _get_program()
    print("program built OK")


# revision 36
# speedup vs baseline: 1.1077x; 1.0567x over previous
"""Trainium2 Bass kernel for nn_Attention_82540681494971.

Spatial self-attention block (LDM AttnBlock style, unscaled):
  qkv = conv1x1(x);  s = q^T k  [n x n] per (b,head);  attn = softmax(s, axis=-1)
  out[d,m] = sum_n v[d,n] attn[n,m];  y = conv1x1(out)

Shapes: B=4, C=64, H=W=64 -> n=4096 tokens, HEAD=4, d=16.

Sharding: 8 cores, core c handles batch b=c//2 and heads (0,1) if c%2==0
else (2,3). Each core computes a partial projection output over its two
heads' channels; host sums the two partials per batch and adds proj bias.

Key algebra: attn[n,m] = E[n,m]/rowsum[n] with E=exp(s). Since the AV
contraction runs over n (the softmax row index), fold 1/rowsum into v:
  out[d,m] = sum_n (v[d,n]*rinv[n]) E[n,m]
so the big E matrix never needs normalizing.

Engine plan (ACT is the hard floor: 33.5M exp/core @ 1 elem/cyc/lane):
 - qkv: host pre-splits x and W into bf16 hi/lo; 2 bf16 matmuls per
   512-token chunk produce q,k,v for BOTH heads at once (M=96).
 - scores: exact-fp32-grade via a 3-term bf16 split (drop lo*lo):
     s = [q_hi;q_lo;q_hi]^T [k_hi;k_hi;k_lo]   (K=48 stacked)
 - AV: 8 persistent col-tiled accumulation chains per head live in 2
   PSUM banks for the whole head (tile_position cols 0/32/64/96); the
   first matmul of each bank is widened to M=112 with a zero-padded
   vts so the whole bank gets defined values + has_written bits.
 - exp: 3 chunked ACTIVATEs (1536/1536/1024) per 128-row block with
   accum_out giving the rowsum; ping-pong over 2x3 PSUM banks.
 - proj: outsb (bf16) stacked for both heads, single K=64 bf16 matmul.
"""

import numpy as np
import ml_dtypes
from contextlib import ExitStack

import concourse.bass as bass
import concourse.mybir as mybir
import concourse.tile as tile
from concourse import bacc
from concourse.bass import ts, ds
from concourse.bass_utils import run_bass_kernel_spmd

F32 = mybir.dt.float32
BF16 = mybir.dt.bfloat16
AF = mybir.ActivationFunctionType
BF = ml_dtypes.bfloat16

B, C, HEAD, D = 4, 64, 4, 16
N = 4096          # tokens = H*W
NT = 128          # n-tile (partition) size
NTILES = N // NT  # 32
MC = 512          # matmul free-dim chunk
# scores/exp PSUM chunking: small chunk LAST -- the freed buffer's next
# refill then has the longest runway before its EXP needs it
SCH = (1536, 1536, 1024)


def _body(tc, y, t1, t2, whiB, whlB, wv1, wv2, wps0, wps1):
    nc = tc.nc
    ctx = ExitStack()
    with ctx:
        pp = ctx.enter_context(tc.tile_pool(name="persist", bufs=1))
        cp = ctx.enter_context(tc.tile_pool(name="consts", bufs=1))

        # ---- persistent SBUF ----
        t1_t = pp.tile([C + 1, N], BF16)   # [x_hi; ones]
        t2_t = pp.tile([2 * C, N], BF16)   # [x_lo; x_hi]
        # score stacks mirrored at partition 64: consecutive score matmuls
        # alternate PE row groups so each LDWEIGHTS hides under the other
        # group's stream (without this every matmul pays ldw serially).
        qsp = pp.tile([64 + 3 * D, 2 * N], BF16)  # [q_hi; q_lo; q_hi] x2
        ksp = pp.tile([64 + 3 * D, 2 * N], BF16)  # [k_hi; k_hi; k_lo] x2
        hi_t = pp.tile([96, N], BF16)      # bf16 of qkv psum (q0 q1 k0 k1 v0 v1)
        lo_t = pp.tile([C, N], BF16)       # residual for q,k rows
        vT = pp.tile([NT, 32 * NTILES], BF16)  # per n-tile [128, 32] (v h0|h1)
        # AV results, one [112, 1024] buffer per head; av tile ti -> cols
        # ti*512, m-chunk = 4*ti + strip/32 on partition rows strip..strip+16
        out_sb = [pp.tile([112, 1024], BF16, tag=f"osb{h}", name=f"osb{h}")
                  for h in range(2)]
        y_sb = pp.tile([C, N], F32)
        wtile = pp.tile([NT, MC], BF16)    # PE warm-up fodder
        # zero-padded vts tiles (cols 16-111 stay 0 for the widened first
        # AV matmul); memset early so phase-1 gpsimd work isn't blocked
        # behind the gpsimd-dispatched assembly DMAs
        vpads = [pp.tile([NT, 112], BF16, name=f"vpad{h}") for h in range(2)]
        nc.gpsimd.memset(vpads[0][:], 0.0)
        nc.gpsimd.memset(vpads[1][:], 0.0)

        # ---- constants ----
        whi_t = cp.tile([C + 1, 96], BF16)
        whl_t = cp.tile([2 * C, 96], BF16)
        wv1_t = cp.tile([C + 1, 32], BF16)   # [wv_lo; bv]
        wv2_t = cp.tile([2 * C, 32], BF16)   # [wv_hi; wv_hi]
        wps_t = [cp.tile([112, C], BF16, name=f"wps{h}") for h in range(2)]
        nc.gpsimd.memset(wtile[:], 0.0)
        for c in range(2):
            nc.sync.dma_start(t1_t[:, ts(c, 4 * MC)], t1[:, ts(c, 4 * MC)])
            nc.sync.dma_start(t2_t[:, ts(c, 4 * MC)], t2[:, ts(c, 4 * MC)])
            if c == 0:
                nc.sync.dma_start(whi_t[:], whiB[:])
                nc.sync.dma_start(whl_t[:], whlB[:])
                nc.sync.dma_start(wv1_t[:], wv1[:])
                nc.sync.dma_start(wv2_t[:], wv2[:])
        nc.gpsimd.dma_start(wps_t[0][:], wps0[:])
        nc.gpsimd.dma_start(wps_t[1][:], wps1[:])

        # ---- phase 0: fused qkv + bf16 hi/lo split ----
        with tc.tile_pool(name="p0psum", bufs=2, space="PSUM") as p0:
            # warm the PE's HAM clock gate while DMAs land: ~5us of dummy
            # matmuls with no DMA dependency (wtile is memset on-chip)
            for w in range(8):
                wps = p0.tile([NT, MC], F32, tag="warm", bufs=2,
                              name=f"warm{w}")
                nc.tensor.matmul(wps[:], wtile[:, ds(0, NT)], wtile[:],
                                 start=True, stop=True)
            for g in range(4):  # 1024-wide groups: fewer, larger evac ops
                ps = p0.tile([96, 2 * MC], F32, tag="p0", name=f"qkv{g}")
                for ci in range(2):
                    c = 2 * g + ci
                    nc.tensor.matmul(ps[:, ts(ci, MC)], whi_t[:],
                                     t1_t[:, ts(c, MC)], start=True, stop=False)
                    nc.tensor.matmul(ps[:, ts(ci, MC)], whl_t[:],
                                     t2_t[:, ts(c, MC)], start=False, stop=True)
                # hi on ACT (idle in phase 0), lo on DVE
                nc.scalar.copy(hi_t[:, ts(g, 2 * MC)], ps[:])
                nc.vector.tensor_sub(lo_t[:, ts(g, 2 * MC)], ps[ds(0, C), :],
                                     hi_t[ds(0, C), ts(g, 2 * MC)])
            # v for n-tiles 0-7 on the PE (token-partition form); only these
            # are needed early, the rest transpose in phase 1
            for nt in range(8):
                psv = p0.tile([NT, 32], F32, tag="pv", bufs=2,
                              name=f"psv{nt}")
                nc.tensor.matmul(psv[:], t2_t[:, ds(nt * NT, NT)],
                                 wv2_t[:], start=True, stop=False)
                nc.tensor.matmul(psv[:], t1_t[:, ds(nt * NT, NT)],
                                 wv1_t[:], start=False, stop=True)
                nc.vector.tensor_copy(vT[:, ts(nt, 32)], psv[:])

            # assembly into the mirrored K=48 score stacks (SBUF->SBUF DMA;
            # engines can't write partition base 16). SBUF->SBUF DMA runs at
            # only ~110GB/s serially on the dispatching queue, so it can't be
            # made fast -- it must be HIDDEN: emit in dependency-priority
            # order (k h0 quarter 0 first, unblocking the first EXPs) and
            # let the rest stream in behind the early attention steps.
            def asm(eng, t, h, hf):
                dsl = ds(h * N + hf * 2048, 2048)
                sl = ds(hf * 2048, 2048)
                if t == "k":
                    row = 32 + h * D
                    blocks = ((0, hi_t), (D, hi_t), (2 * D, lo_t))
                else:
                    row = h * D
                    blocks = ((0, hi_t), (D, lo_t), (2 * D, hi_t))
                dst = ksp if t == "k" else qsp
                for boff, src in blocks:
                    eng.dma_start(dst[ds(boff, D), dsl], src[ds(row, D), sl])
                # mirror the whole 48-row stack to partition 64 in one op
                eng.dma_start(dst[ds(64, 3 * D), dsl], dst[ds(0, 3 * D), dsl])

            # head 0 on the sync queue in priority order (the sequencer
            # costs ~0.6us per DMA serially); all of head 1 on the gpsimd
            # software DGE, whose dispatch overlaps sync's
            asm(nc.sync, "k", 0, 0)
            asm(nc.sync, "q", 0, 0)
            asm(nc.sync, "k", 0, 1)
            asm(nc.sync, "q", 0, 1)
            for hf in range(2):
                asm(nc.gpsimd, "k", 1, hf)
                asm(nc.gpsimd, "q", 1, hf)

        # ---- phase 1: attention, software-pipelined ----
        # Per step (h, nt): emit this n-tile's score matmuls + exp, woven
        # with the AV matmuls of the previous step. AV chains accumulate in
        # PSUM across the whole head (col-tiled 4-way, 2 banks per head).
        with (
            tc.tile_pool(name="ep", bufs=3) as ep,
            tc.tile_pool(name="rp", bufs=4) as rp,
            tc.tile_pool(name="vp", bufs=3) as vp,
            tc.tile_pool(name="sapsum", bufs=2, space="PSUM") as sp,
            tc.tile_pool(name="avpsum", bufs=1, space="PSUM") as avp,
        ):
            av_state = {}

            def emit_evac(h, engines):
                av_t, _ = av_state[h]
                for ti in range(2):
                    dst = out_sb[h][:, ts(ti, MC)]
                    if engines == "both" and ti == 1:
                        nc.scalar.copy(dst, av_t[ti][:])
                    else:
                        nc.vector.tensor_copy(dst, av_t[ti][:])

            prev = None
            for s in range(64):
                h, nt = divmod(s, NTILES)
                if nt == 0:
                    av_t = [avp.tile([112, MC], F32, tag=f"av{i}",
                                     name=f"av{i}h{h}") for i in range(2)]
                    av_state[h] = (av_t, vpads[h])

                e_t = ep.tile([NT, N], BF16, tag="e", name=f"e{h}_{nt}")
                rsp = rp.tile([NT, 4], F32, tag="rs", name="rsp")

                chains = []
                if prev is not None:
                    ph, pnt, pe_t, pvts = prev
                    pav_t, pvpad = av_state[ph]

                    def mk(mc, pnt=pnt, pe_t=pe_t, pvts=pvts,
                           pav_t=pav_t, pvpad=pvpad):
                        def go():
                            ti, strip = mc // 4, 32 * (mc % 4)
                            if pnt == 0 and mc % 4 == 0:
                                # widened first matmul: writes the vts
                                # product on partitions 0-15 and zeros on
                                # 16-111, claiming the whole bank.
                                nc.tensor.matmul(
                                    pav_t[ti][:, :], pvpad[:, ds(0, 112)],
                                    pe_t[:, ts(mc, MC)],
                                    start=True, stop=False,
                                    skip_group_check=True)
                            else:
                                nc.tensor.matmul(
                                    pav_t[ti][ds(strip, D), :], pvts,
                                    pe_t[:, ts(mc, MC)],
                                    start=False, stop=(pnt == NTILES - 1),
                                    tile_position=(0, strip),
                                    skip_group_check=True)
                        return go

                    chains = [mk(mc) for mc in range(8)]

                # weave: score chunks c0+c1 first (they gate the EXP chain),
                # then the prev step's AV block (gated by its vts), then c2.
                off = 0
                for ci, csz in enumerate(SCH):
                    s_ps = sp.tile([NT, max(SCH)], F32, tag="sa", name="s_ps")
                    for i in range(csz // MC):
                        b0 = 64 if (off // MC + i) % 2 else 0
                        nc.tensor.matmul(
                            s_ps[:, ts(i, MC)],
                            qsp[ds(b0, 3 * D), ds(h * N + nt * NT, NT)],
                            ksp[ds(b0, 3 * D), ds(h * N + off + i * MC, MC)],
                            start=True, stop=True)
                    nc.scalar.activation(
                        e_t[:, ds(off, csz)], s_ps[:, ds(0, csz)],
                        AF.Exp, accum_out=rsp[:, ds(ci, 1)])
                    off += csz
                    if ci == 1:
                        for _ in range(6):
                            if chains:
                                chains.pop(0)()
                while chains:
                    chains.pop(0)()
                if s < 24:
                    # v for n-tile s+8 via xbar transpose (serial on the
                    # sync engine, ~1.2us each -- one per step is free here)
                    nc.sync.dma_start(vT[:, ts(s + 8, 32)],
                                      hi_t[ds(64, 32), ts(s + 8, NT)],
                                      transpose=True)

                rs = rp.tile([NT, 1], F32, tag="r1", name="rs1")
                rinv = rp.tile([NT, 1], F32, tag="ri", name="rinv")
                nc.vector.reduce_sum(rs[:], rsp[:, ds(0, 3)],
                                     axis=mybir.AxisListType.X)
                nc.vector.reciprocal(rinv[:], rs[:])
                if nt == 0:
                    vts = av_state[h][1][:, ds(0, D)]
                else:
                    vts_t = vp.tile([NT, D], BF16, tag="vts",
                                    name=f"vts{h}_{nt}")
                    vts = vts_t[:]
                nc.vector.tensor_scalar_mul(
                    vts, vT[:, ds(nt * 32 + h * D, D)], rinv[:])
                prev = (h, nt, e_t, vts)

                if s == NTILES:
                    # AV(h0, 31) was just woven above; drain head-0 chains
                    emit_evac(0, "vector")

            # ---- tail: flush AV(h1, 31), drain, project ----
            ph, pnt, pe_t, pvts = prev
            pav_t, _ = av_state[ph]
            for mc in range(8):
                ti, strip = mc // 4, 32 * (mc % 4)
                nc.tensor.matmul(
                    pav_t[ti][ds(strip, D), :], pvts, pe_t[:, ts(mc, MC)],
                    start=False, stop=True, tile_position=(0, strip),
                    skip_group_check=True)
            emit_evac(1, "both")

            # y[:, mc] = sum_h wps_h^T out_h[:, mc]; wps replicas at every
            # 32-strip let lhsT/rhs share the strip's base partition.
            # yp rotates through 4 PSUM slots (sa x2 + the freed av banks)
            # so the MM -> copy -> DMA chain pipelines 4 deep.
            for mc in range(8):
                ti, strip = mc // 4, 32 * (mc % 4)
                tag = ("sa", "av0", "av1")[mc % 3]
                yp = sp.tile([C, MC], F32, tag="sa", name=f"yp{mc}") \
                    if tag == "sa" else \
                    avp.tile([C, MC], F32, tag=tag, name=f"yp{mc}")
                for hh in range(2):
                    nc.tensor.matmul(
                        yp[:], wps_t[hh][ds(strip, D), :],
                        out_sb[hh][ds(strip, D), ts(ti, MC)],
                        start=(hh == 0), stop=(hh == 1),
                        tile_position=(strip, 0))
                if mc % 2 == 0:
                    nc.vector.tensor_copy(y_sb[:, ts(mc, MC)], yp[:])
                else:
                    nc.scalar.copy(y_sb[:, ts(mc, MC)], yp[:])
                nc.sync.dma_start(y[:, ts(mc, MC)], y_sb[:, ts(mc, MC)])


_PROGRAM = None


def _get_program():
    global _PROGRAM
    if _PROGRAM is None:
        nc = bacc.Bacc("TRN2", target_bir_lowering=False, debug=False,
                       num_devices=8)
        t1 = nc.dram_tensor("t1", [C + 1, N], BF16, kind="ExternalInput").ap()
        t2 = nc.dram_tensor("t2", [2 * C, N], BF16, kind="ExternalInput").ap()
        whiB = nc.dram_tensor("whiB", [C + 1, 96], BF16, kind="ExternalInput").ap()
        whlB = nc.dram_tensor("whlB", [2 * C, 96], BF16, kind="ExternalInput").ap()
        wv1 = nc.dram_tensor("wv1", [C + 1, 32], BF16, kind="ExternalInput").ap()
        wv2 = nc.dram_tensor("wv2", [2 * C, 32], BF16, kind="ExternalInput").ap()
        wps0 = nc.dram_tensor("wps0", [112, C], BF16, kind="ExternalInput").ap()
        wps1 = nc.dram_tensor("wps1", [112, C], BF16, kind="ExternalInput").ap()
        y = nc.dram_tensor("y", [C, N], F32, kind="ExternalOutput").ap()
        with tile.TileContext(nc) as tc:
            _body(tc, y, t1, t2, whiB, whlB, wv1, wv2, wps0, wps1)
        nc.compile()
        _PROGRAM = nc
    return _PROGRAM


def _make_in_maps(x, qkv_w, qkv_b, proj_w, proj_b=None):
    x = np.asarray(x, dtype=np.float32)
    qkv_w = np.asarray(qkv_w, dtype=np.float32)
    qkv_b = np.asarray(qkv_b, dtype=np.float32)
    proj_w = np.asarray(proj_w, dtype=np.float32)

    # per-batch tensors (shared by the 2 cores of a batch)
    t1s, t2s = [], []
    for b in range(B):
        xf = x[b].reshape(C, N)
        xh = xf.astype(BF)
        xl = (xf - xh.astype(np.float32)).astype(BF)
        t1s.append(np.ascontiguousarray(
            np.concatenate([xh, np.ones((1, N), BF)], axis=0)))
        t2s.append(np.ascontiguousarray(np.concatenate([xl, xh], axis=0)))

    in_maps = []
    for core in range(8):
        b = core // 2
        h0 = 2 * (core % 2)
        heads = (h0, h0 + 1)

        # weight stack cols: [q_h0, q_h1, k_h0, k_h1, v_h0, v_h1]
        rows = []
        bias = []
        for blk in range(3):
            for h in heads:
                r = slice(blk * C + h * D, blk * C + (h + 1) * D)
                rows.append(qkv_w[r, :])
                bias.append(qkv_b[r])
        Wsel = np.concatenate(rows, axis=0)          # [96, 64]
        bsel = np.concatenate(bias, axis=0)          # [96]
        Whi = Wsel.astype(BF)
        Wlo = (Wsel - Whi.astype(np.float32)).astype(BF)
        whiB = np.zeros((C + 1, 96), BF)
        whiB[:C] = Whi.T
        whiB[C] = bsel.astype(BF)
        whlB = np.ascontiguousarray(
            np.concatenate([Whi.T.astype(BF), Wlo.T.astype(BF)], axis=0))
        # v weights for the early token-partition v matmuls (cols 64-95 of
        # the stack are the v heads)
        wv1B = np.zeros((C + 1, 32), BF)
        wv1B[:C] = Wlo.T[:, 64:96]
        wv1B[C] = bsel[64:96].astype(BF)
        wv2B = np.ascontiguousarray(
            np.concatenate([Whi.T[:, 64:96], Whi.T[:, 64:96]], axis=0))

        # proj weights, replicated at every 32-partition strip so the proj
        # matmul's lhsT base partition matches its rhs strip
        wpss = []
        for h in heads:
            w = np.zeros((112, C), BF)
            blk = proj_w[:, h * D:(h + 1) * D].T.astype(BF)
            for strip in range(4):
                w[strip * 32:strip * 32 + D, :] = blk
            wpss.append(w)

        in_maps.append({
            "t1": t1s[b],
            "t2": t2s[b],
            "whiB": whiB,
            "whlB": whlB,
            "wv1": wv1B,
            "wv2": wv2B,
            "wps0": wpss[0],
            "wps1": wpss[1],
        })
    return in_maps


def run_cores(inputs, **kw):
    """Compile+run on the 8 cores; returns BassKernelResults."""
    nc = _get_program()
    in_maps = _make_in_maps(**inputs)
    return run_bass_kernel_spmd(nc, in_maps, list(range(8)), **kw)


def kernel(x, qkv_w, qkv_b, proj_w, proj_b):
    res = run_cores(dict(x=x, qkv_w=qkv_w, qkv_b=qkv_b,
                         proj_w=proj_w, proj_b=proj_b))
    proj_b = np.asarray(proj_b, dtype=np.float32)
    parts = [np.asarray(r["y"], dtype=np.float32) for r in res.results]
    out = np.empty((B, C, N), np.float32)
    for b in range(B):
        out[b] = parts[2 * b] + parts[2 * b + 1] + proj_b[:, None]
    return out.reshape(B, C, 64, 64)


if __name__ == "__main__":
    _get_program()
    print("program built OK")
